# revision 18
# baseline (speedup 1.0000x reference)
"""Trainium2 Bass kernel for 2-layer GraphSAGE (mean aggregation) on 8 NeuronCores.

Math: with M = mean-aggregation operator (D^-1 A), the reference is
    h  = relu(x @ W1 + b1)
    h1 = (M h) Wl1 + bl1 + h Wr1
    h2 = (M h1) Wl2 + bl2 + h1 Wr2
    out = h2 @ W2 + b2
Everything after the relu is linear, so fold:
    out = M(M(h C2)) + M(h C1) + h C0 + r*c_r + c_1
with C2 = Wl1 Wl2 W2, C1 = (Wr1 Wl2 + Wl1 Wr2) W2, C0 = Wr1 Wr2 W2,
c_r = bl1 Wl2 W2, c_1 = (bl2 + bl1 Wr2) W2 + b2, r = (deg > 0).
The two aggregation passes therefore run on 3-feature vectors.

Distribution: NC n owns src-shard n (12500 nodes; computes h locally).
Its incident edges are grouped by dst-range -> Q7 core, chunked and sorted
by dst.  Per chunk: GPSIMD ap_gather (feature-per-partition tables, int16
src-local indices) -> DVE cumulative-sum scan -> GPSIMD gather at segment
ends -> shifted subtract = per-dst sums.  A PE one-hot matmul compacts the
(group, feature) partitions, and partial sums are ReduceScattered across
the 8 NCs (dst-shard n -> NC n), divided by degree on device, and fed to
pass 2.  Host-side work is only integer graph partitioning / index layout.
"""
import numpy as np

# ---- problem constants (hardcoded per contract) ----
N = 100000
E = 6400000
IN_F = 128
HID = 10
OUT = 3

NCN = 8             # NeuronCores
QC = 8              # Q7 cores per NC
NSH = N // NCN      # 12500 nodes per shard
K_CH = 8            # chunks per (NC, q7core)
D_CHUNK = -(-NSH // K_CH)          # 1042 dsts per chunk
D_STRIP = K_CH * D_CHUNK           # 12504
NE = ((D_CHUNK + 1 + 15) // 16) * 16   # 1056 ends entries per chunk
NSH_TAB = ((NSH + 16 + 15) // 16) * 16  # 12528 table width
SENT = NSH + 6                      # sentinel (zero) table column
XPAD = ((NSH + 511) // 512) * 512   # 12800 padded x rows (512-row groups)
F32 = "float32"

_prog_cache = {}


def _build_program(C_CAP):
    from contextlib import ExitStack
    import concourse.bacc as bacc
    import concourse.tile as tile
    import concourse.mybir as mybir
    from concourse.masks import make_identity

    f32 = mybir.dt.float32
    i16 = mybir.dt.int16
    AL = mybir.AluOpType
    AF = mybir.ActivationFunctionType

    nc = bacc.Bacc("TRN2", target_bir_lowering=False, debug=False,
                   num_devices=NCN)

    # ---- I/O ----
    x_in = nc.dram_tensor("x_sh", [XPAD, IN_F], f32, kind="ExternalInput")
    eidx_in = nc.dram_tensor("eidx", [K_CH, 128, C_CAP // 16], i16, kind="ExternalInput")
    eend_in = nc.dram_tensor("eends", [K_CH, 128, NE // 16], i16, kind="ExternalInput")
    deg_in = nc.dram_tensor("deg3", [3, D_STRIP], f32, kind="ExternalInput")
    sel_in = nc.dram_tensor("selAB", [128, 48], f32, kind="ExternalInput")
    rep72_in = nc.dram_tensor("rep72", [3, 72], f32, kind="ExternalInput")
    W1_in = nc.dram_tensor("W1", [IN_F, HID], f32, kind="ExternalInput")
    b1_in = nc.dram_tensor("b1c", [HID, 1], f32, kind="ExternalInput")
    Wl2_in = nc.dram_tensor("Wl2", [HID, HID], f32, kind="ExternalInput")
    Wr2_in = nc.dram_tensor("Wr2", [HID, HID], f32, kind="ExternalInput")
    W2_in = nc.dram_tensor("W2", [HID, OUT], f32, kind="ExternalInput")
    Wl1T_in = nc.dram_tensor("Wl1T", [HID, HID], f32, kind="ExternalInput")
    Wr1T_in = nc.dram_tensor("Wr1T", [HID, HID], f32, kind="ExternalInput")
    bl1_in = nc.dram_tensor("bl1c", [HID, 1], f32, kind="ExternalInput")
    bl2_in = nc.dram_tensor("bl2c", [HID, 1], f32, kind="ExternalInput")
    b2_in = nc.dram_tensor("b2c", [OUT, 1], f32, kind="ExternalInput")
    out_ext = nc.dram_tensor("outT", [3, D_STRIP], f32, kind="ExternalOutput")

    with tile.TileContext(nc) as tc:
        es = ExitStack()
        with es:
            dram = es.enter_context(tc.tile_pool(name="dram", bufs=1, space="DRAM"))
            p_small = es.enter_context(tc.tile_pool(name="small", bufs=1))

            gc0_d = dram.tile([3, D_STRIP], f32)
            WA = D_STRIP // 24          # 521: [3, D_STRIP] viewed as [72, 521]

            def v72(ap):
                return ap.rearrange("f (a b) -> (f a) b", a=24)
            bounceA = dram.tile([NCN, 3, D_STRIP], f32)
            bounceB = dram.tile([NCN, 3, D_STRIP], f32)
            bounceC = dram.tile([NCN, 3, D_STRIP], f32)
            rsA = dram.tile([3, D_STRIP], f32)
            rsB = dram.tile([3, D_STRIP], f32)
            rsC = dram.tile([3, D_STRIP], f32)

            sel = p_small.tile([128, 48], f32)
            nc.sync.dma_start(out=sel[:], in_=sel_in[:])

            # ---- phase 0: folded weight matrices (tiny PE matmuls) ----
            w1 = p_small.tile([IN_F, HID], f32)
            wl2 = p_small.tile([HID, HID], f32)
            wr2 = p_small.tile([HID, HID], f32)
            w2 = p_small.tile([HID, OUT], f32)
            wl1t = p_small.tile([HID, HID], f32)
            wr1t = p_small.tile([HID, HID], f32)
            b1c = p_small.tile([HID, 1], f32)
            bl1c = p_small.tile([HID, 1], f32)
            bl2c = p_small.tile([HID, 1], f32)
            b2c = p_small.tile([OUT, 1], f32)
            for t, src in [(w1, W1_in), (wl2, Wl2_in), (wr2, Wr2_in),
                           (w2, W2_in), (wl1t, Wl1T_in), (wr1t, Wr1T_in),
                           (b1c, b1_in), (bl1c, bl1_in), (bl2c, bl2_in),
                           (b2c, b2_in)]:
                nc.sync.dma_start(out=t[:], in_=src[:])

            rec_d = dram.tile([3, D_STRIP], f32)
            with tc.tile_pool(name="rec0", bufs=1) as p_rec:
                rt0 = p_rec.tile([72, WA], f32)
                nc.scalar.dma_start(out=rt0[:], in_=v72(deg_in[:]))
                nc.vector.tensor_scalar_max(out=rt0[:], in0=rt0[:], scalar1=1.0)
                nc.vector.reciprocal(out=rt0[:], in_=rt0[:])
                nc.scalar.dma_start(out=v72(rec_d[:]), in_=rt0[:])

            p_ps0 = es.enter_context(tc.tile_pool(name="psum0", bufs=1, space="PSUM"))

            def mm(lhsT, rhs, m, n_, accum=None):
                """matmul into fresh psum, copy to fresh small sbuf tile."""
                ps = p_ps0.tile([m, n_], f32, space="PSUM", tag="ps0")
                if accum is None:
                    nc.tensor.matmul(out=ps[:], lhsT=lhsT, rhs=rhs, start=True, stop=True)
                else:
                    nc.tensor.matmul(out=ps[:], lhsT=lhsT, rhs=rhs, start=True, stop=False)
                    nc.tensor.matmul(out=ps[:], lhsT=accum[0], rhs=accum[1], start=False, stop=True)
                sb = p_small.tile([m, n_], f32, tag=f"mm_{m}x{n_}_{nc.next_id()}")
                nc.vector.tensor_copy(out=sb[:], in_=ps[:])
                return sb

            s2 = mm(wl2[:], wl1t[:], HID, HID)                     # (Wl1 Wl2)^T
            rt = mm(wl2[:], wr1t[:], HID, HID, accum=(wr2[:], wl1t[:]))  # R^T
            s0 = mm(wr2[:], wr1t[:], HID, HID)                     # (Wr1 Wr2)^T
            ccc = p_small.tile([HID, 9], f32)
            for j, lh in [(0, rt), (3, s2), (6, s0)]:
                ps = p_ps0.tile([HID, OUT], f32, space="PSUM", tag="ps0")
                nc.tensor.matmul(out=ps[:], lhsT=lh[:], rhs=w2[:], start=True, stop=True)
                nc.vector.tensor_copy(out=ccc[:, j:j + 3], in_=ps[:])
            t1 = mm(wl2[:], bl1c[:], HID, 1)
            crs = mm(w2[:], t1[:], OUT, 1)                          # c_r [3,1]
            u = mm(wr2[:], bl1c[:], HID, 1)
            nc.vector.tensor_tensor(out=u[:], in0=u[:], in1=bl2c[:], op=AL.add)
            c1s = mm(w2[:], u[:], OUT, 1)                           # pre b2
            nc.vector.tensor_tensor(out=c1s[:], in0=c1s[:], in1=b2c[:], op=AL.add)

            # ---- phase 1: h = relu(x W1 + b1); gc = [hC1 | hC2 | hC0] ----
            p_tab = es.enter_context(tc.tile_pool(name="tab", bufs=1))
            tab = p_tab.tile([128, NSH_TAB], f32)
            _pad0 = p_tab.tile([128, 1024], f32)   # shift agg pools +4KB/partition
            nc.scalar.memzero(tab[:])
            with tc.tile_pool(name="lin1", bufs=2) as p_lin, \
                 tc.tile_pool(name="lin1gc", bufs=1) as p_gc, \
                 tc.tile_pool(name="lin1ps", bufs=2, space="PSUM") as p_lps:
                ident = p_small.tile([128, 128], f32)
                make_identity(nc, ident[:])
                gcf = p_gc.tile([9, XPAD], f32)
                n_grp = XPAD // 512
                for g in range(n_grp):
                    xt4 = p_lin.tile([128, 4, 128], f32, tag="xt4")
                    # rows 512g..512g+512 of x -> [p, t, f]
                    nc.sync.dma_start(
                        out=xt4[:],
                        in_=x_in[:].rearrange("(a t p) f -> a p t f", a=XPAD // 512, t=4, p=128)[g])
                    tps = p_lps.tile([128, 512], f32, space="PSUM", tag="tps")
                    for t in range(4):
                        nc.tensor.transpose(out=tps[:, t * 128:(t + 1) * 128],
                                            in_=xt4[:, t, :], identity=ident[:])
                    xtb = p_lin.tile([128, 512], f32, tag="xtb")
                    nc.vector.tensor_copy(out=xtb[:], in_=tps[:])
                    hps = p_lps.tile([HID, 512], f32, space="PSUM", tag="hps")
                    nc.tensor.matmul(out=hps[:], lhsT=w1[:], rhs=xtb[:], start=True, stop=True)
                    hb = p_lin.tile([HID, 512], f32, tag="hb")
                    nc.scalar.activation(out=hb[:], in_=hps[:], func=AF.Relu,
                                         bias=b1c[:], scale=1.0)
                    gps = p_lps.tile([9, 512], f32, space="PSUM", tag="gps")
                    nc.tensor.matmul(out=gps[:], lhsT=ccc[:], rhs=hb[:], start=True, stop=True)
                    nc.vector.tensor_copy(out=gcf[:, g * 512:(g + 1) * 512], in_=gps[:])
                # distribute into gather table (per 16-partition group) + gc0 out
                for g in range(QC):
                    eng = nc.sync if g % 2 == 0 else nc.scalar
                    eng.dma_start(out=tab[16 * g:16 * g + 6, 0:NSH],
                                  in_=gcf[0:6, 0:NSH])
                nc.sync.dma_start(out=gc0_d[:], in_=gcf[6:9, 0:D_STRIP])

            # ---- aggregation passes (software-pipelined on the GPSIMD queue:
            # chunk k+1's main gather is issued before chunk k's ends gather
            # so the Q7 never idles waiting on the DVE scan) ----
            def agg_pass(bounces):
                """bounces: list of (comp_row_offset, dram_view24) to store."""
                with tc.tile_pool(name="agg_msg", bufs=2) as p_msg, \
                     tc.tile_pool(name="agg_sm", bufs=2) as p_asm, \
                     tc.tile_pool(name="agg_ps", bufs=2, space="PSUM") as p_aps:
                    live = {}

                    def front(k):
                        idx_t = p_asm.tile([128, C_CAP // 16], i16, tag="idx")
                        nc.sync.dma_start(out=idx_t[:], in_=eidx_in[k])
                        end_t = p_asm.tile([128, NE // 16], i16, tag="end")
                        nc.sync.dma_start(out=end_t[:], in_=eend_in[k])
                        msg = p_msg.tile([128, C_CAP], f32, tag="msg")
                        nc.gpsimd.ap_gather(
                            out_ap=msg[:], in_ap=tab[:], idxs_ap=idx_t[:],
                            channels=128, num_elems=NSH_TAB, d=1, num_idxs=C_CAP)
                        nc.vector.tensor_tensor_scan(
                            out=msg[:], data0=msg[:], data1=msg[:], initial=0.0,
                            op0=AL.add, op1=AL.bypass)
                        live[k] = (msg, end_t)

                    def back(k):
                        msg, end_t = live.pop(k)
                        gat = p_asm.tile([128, NE], f32, tag="gat")
                        nc.gpsimd.ap_gather(
                            out_ap=gat[:], in_ap=msg[:], idxs_ap=end_t[:],
                            channels=128, num_elems=C_CAP, d=1, num_idxs=NE)
                        strip = p_asm.tile([128, D_CHUNK], f32, tag="strip")
                        nc.vector.tensor_tensor(
                            out=strip[:], in0=gat[:, 1:1 + D_CHUNK],
                            in1=gat[:, 0:D_CHUNK], op=AL.subtract)
                        comp = p_asm.tile([48, D_CHUNK], f32, tag="comp")
                        for j in range(0, D_CHUNK, 512):
                            w = min(512, D_CHUNK - j)
                            cps = p_aps.tile([48, w], f32, space="PSUM", tag="cps")
                            nc.tensor.matmul(out=cps[:], lhsT=sel[:],
                                             rhs=strip[:, j:j + w], start=True, stop=True)
                            nc.vector.tensor_copy(out=comp[:, j:j + w], in_=cps[:])
                        for off, view24 in bounces:
                            nc.sync.dma_start(
                                out=view24[:, k * D_CHUNK:(k + 1) * D_CHUNK],
                                in_=comp[off:off + 24, :])

                    for k in range(K_CH):
                        front(k)
                        if k >= 1:
                            back(k - 1)
                    back(K_CH - 1)

            vA = bounceA[:].rearrange("g f d -> (g f) d")
            vB = bounceB[:].rearrange("g f d -> (g f) d")
            vC = bounceC[:].rearrange("g f d -> (g f) d")

            agg_pass([(0, vA), (24, vB)])

            rg = [list(range(NCN))]
            nc.gpsimd.collective_compute("ReduceScatter", AL.add, replica_groups=rg,
                                         ins=[bounceA.opt()], outs=[rsA.opt()])

            # build pass-2 table: a2' = rsA * recip, replicated per group
            nc.scalar.memzero(tab[:])
            with tc.tile_pool(name="mid", bufs=2) as p_mid:
                ta = p_mid.tile([3, D_STRIP], f32, tag="wide")
                nc.sync.dma_start(out=ta[:], in_=rsA[:])
                td = p_mid.tile([3, D_STRIP], f32, tag="wide")
                nc.scalar.dma_start(out=td[:], in_=rec_d[:])
                nc.vector.tensor_tensor(out=ta[:], in0=ta[:], in1=td[:], op=AL.mult)
                for g in range(QC):
                    eng = nc.sync if g % 2 == 0 else nc.scalar
                    eng.dma_start(out=tab[16 * g:16 * g + 3, 0:D_STRIP], in_=ta[:])

            # RS of the B partials overlaps pass 2 (collectives are async
            # w.r.t. the issuing queue; rsB is only read by final assembly)
            nc.gpsimd.collective_compute("ReduceScatter", AL.add, replica_groups=rg,
                                         ins=[bounceB.opt()], outs=[rsB.opt()])

            agg_pass([(24, vC)])

            nc.gpsimd.collective_compute("ReduceScatter", AL.add, replica_groups=rg,
                                         ins=[bounceC.opt()], outs=[rsC.opt()])

            # replicate [crs | c1s] to 72 partitions for the [72, WA] final math
            rep72 = p_small.tile([3, 72], f32)
            nc.sync.dma_start(out=rep72[:], in_=rep72_in[:])
            cc2 = p_small.tile([3, 2], f32)
            nc.vector.tensor_copy(out=cc2[:, 0:1], in_=crs[:])
            nc.vector.tensor_copy(out=cc2[:, 1:2], in_=c1s[:])
            ps72 = p_ps0.tile([72, 2], f32, space="PSUM", tag="ps0")
            nc.tensor.matmul(out=ps72[:], lhsT=rep72[:], rhs=cc2[:], start=True, stop=True)
            crsc = p_small.tile([72, 2], f32)
            nc.vector.tensor_copy(out=crsc[:], in_=ps72[:])
            # ---- final assembly: out = (a1+b)*recip + gc0 + r*c_r + c_1 ----
            # all elementwise math on a [72, WA] view (24x the lanes of [3, D_STRIP])
            with tc.tile_pool(name="fin", bufs=3) as p_fin:
                s1 = p_fin.tile([72, WA], f32, tag="fw")
                nc.sync.dma_start(out=s1[:], in_=v72(rsB[:]))
                s2_ = p_fin.tile([72, WA], f32, tag="fw")
                nc.sync.dma_start(out=s2_[:], in_=v72(rsC[:]))
                nc.vector.tensor_tensor(out=s1[:], in0=s1[:], in1=s2_[:], op=AL.add)
                sd = p_fin.tile([72, WA], f32, tag="fw")
                nc.scalar.dma_start(out=sd[:], in_=v72(deg_in[:]))
                sr = p_fin.tile([72, WA], f32, tag="fw")
                nc.sync.dma_start(out=sr[:], in_=v72(rec_d[:]))
                nc.vector.tensor_tensor(out=s1[:], in0=s1[:], in1=sr[:], op=AL.mult)
                # r = (deg > 0); s1 += r * c_r
                nc.vector.tensor_scalar(out=sd[:], in0=sd[:], scalar1=0.0, scalar2=None,
                                        op0=AL.is_gt)
                nc.vector.scalar_tensor_tensor(out=s1[:], in0=sd[:], scalar=crsc[:, 0:1],
                                               in1=s1[:], op0=AL.mult, op1=AL.add)
                sg = p_fin.tile([72, WA], f32, tag="fw")
                nc.sync.dma_start(out=sg[:], in_=v72(gc0_d[:]))
                nc.vector.tensor_tensor(out=s1[:], in0=s1[:], in1=sg[:], op=AL.add)
                nc.scalar.activation(out=s1[:], in_=s1[:], func=AF.Identity,
                                     bias=crsc[:, 1:2], scale=1.0)
                nc.sync.dma_start(out=v72(out_ext[:]), in_=s1[:])

    nc.compile()
    return nc


def _preprocess(inputs):
    """Pure-integer graph partitioning + layout; returns per-core in_maps."""
    x = np.ascontiguousarray(np.asarray(inputs["x"], dtype=np.float32))
    edge_index = np.asarray(inputs["edge_index"])
    src = edge_index[0].astype(np.int64)
    dst = edge_index[1].astype(np.int64)

    n_of_e = src // NSH
    c_of_e = dst // NSH
    k_of_e = (dst % NSH) // D_CHUNK
    d_of_e = (dst % NSH) % D_CHUNK
    src_loc = (src % NSH).astype(np.int16)

    key = ((n_of_e * QC + c_of_e) * K_CH + k_of_e) * np.int64(N) + dst
    order = np.argsort(key, kind="stable")
    so = src_loc[order]
    no = n_of_e[order]
    co = c_of_e[order]
    ko = k_of_e[order]
    do_ = d_of_e[order]

    cell = (no * QC + co) * K_CH + ko
    n_cells = NCN * QC * K_CH
    cnt = np.bincount(cell, minlength=n_cells)
    C_CAP = ((int(cnt.max()) + 1 + 15) // 16) * 16
    cell_start = np.zeros(n_cells + 1, np.int64)
    np.cumsum(cnt, out=cell_start[1:])
    slot = np.arange(E) - cell_start[cell] + 1        # slot 0 = sentinel pad

    eidx = np.full((NCN, K_CH, 128, C_CAP // 16), SENT, np.int16)
    eidx[no, ko, 16 * co + (slot % 16), slot // 16] = so

    dcell = cell * D_CHUNK + do_
    cnt_d = np.bincount(dcell, minlength=n_cells * D_CHUNK).reshape(
        NCN, QC, K_CH, D_CHUNK)
    ce = np.cumsum(cnt_d, axis=3).astype(np.int16)
    ends_ext = np.zeros((NCN, QC, K_CH, NE), np.int16)
    ends_ext[:, :, :, 1:D_CHUNK + 1] = ce
    ends_ext[:, :, :, D_CHUNK + 1:] = ce[:, :, :, -1:]
    eends = np.zeros((NCN, K_CH, 128, NE // 16), np.int16)
    for c in range(QC):
        w = ends_ext[:, c].reshape(NCN, K_CH, NE // 16, 16)
        eends[:, :, 16 * c:16 * c + 16, :] = np.swapaxes(w, 2, 3)

    deg = np.bincount(dst, minlength=N).astype(np.float32)
    deg3 = np.zeros((NCN, 3, D_STRIP), np.float32)
    for n in range(NCN):
        deg3[n, :, :NSH] = deg[n * NSH:(n + 1) * NSH][None, :]

    sel = np.zeros((128, 48), np.float32)
    for g in range(QC):
        for f in range(3):
            sel[16 * g + 3 + f, 3 * g + f] = 1.0       # A set: C2/a2 rows
            sel[16 * g + f, 24 + 3 * g + f] = 1.0      # B set: C1/a1 rows

    rep72 = np.zeros((3, 72), np.float32)
    for f in range(3):
        rep72[f, 24 * f:24 * (f + 1)] = 1.0

    W1 = np.ascontiguousarray(np.asarray(inputs["W1"], np.float32))
    wmap = {
        "W1": W1,
        "rep72": rep72,
        "b1c": np.ascontiguousarray(np.asarray(inputs["b1"], np.float32).reshape(HID, 1)),
        "Wl2": np.ascontiguousarray(np.asarray(inputs["Wl2"], np.float32)),
        "Wr2": np.ascontiguousarray(np.asarray(inputs["Wr2"], np.float32)),
        "W2": np.ascontiguousarray(np.asarray(inputs["W2"], np.float32)),
        "Wl1T": np.ascontiguousarray(np.asarray(inputs["Wl1"], np.float32).T),
        "Wr1T": np.ascontiguousarray(np.asarray(inputs["Wr1"], np.float32).T),
        "bl1c": np.ascontiguousarray(np.asarray(inputs["bl1"], np.float32).reshape(HID, 1)),
        "bl2c": np.ascontiguousarray(np.asarray(inputs["bl2"], np.float32).reshape(HID, 1)),
        "b2c": np.ascontiguousarray(np.asarray(inputs["b2"], np.float32).reshape(OUT, 1)),
    }

    in_maps = []
    for n in range(NCN):
        xs = np.zeros((XPAD, IN_F), np.float32)
        xs[:NSH] = x[n * NSH:(n + 1) * NSH]
        m = {
            "x_sh": xs,
            "eidx": np.ascontiguousarray(eidx[n]),
            "eends": np.ascontiguousarray(eends[n]),
            "deg3": np.ascontiguousarray(deg3[n]),
            "selAB": sel,
        }
        m.update(wmap)
        in_maps.append(m)
    return in_maps, C_CAP


def kernel(**inputs) -> np.ndarray:
    from concourse.bass_utils import run_bass_kernel_spmd

    in_maps, C_CAP = _preprocess(inputs)
    if C_CAP not in _prog_cache:
        _prog_cache[C_CAP] = _build_program(C_CAP)
    nc = _prog_cache[C_CAP]
    res = run_bass_kernel_spmd(nc, in_maps, core_ids=list(range(NCN)))
    out = np.empty((N, OUT), np.float32)
    for n in range(NCN):
        out[n * NSH:(n + 1) * NSH] = res.results[n]["outT"].T[:NSH]
    return out



# revision 21
# speedup vs baseline: 1.0011x; 1.0011x over previous
"""Trainium2 Bass kernel for 2-layer GraphSAGE (mean aggregation) on 8 NeuronCores.

Math: with M = mean-aggregation operator (D^-1 A), the reference is
    h  = relu(x @ W1 + b1)
    h1 = (M h) Wl1 + bl1 + h Wr1
    h2 = (M h1) Wl2 + bl2 + h1 Wr2
    out = h2 @ W2 + b2
Everything after the relu is linear, so fold:
    out = M(M(h C2)) + M(h C1) + h C0 + r*c_r + c_1
with C2 = Wl1 Wl2 W2, C1 = (Wr1 Wl2 + Wl1 Wr2) W2, C0 = Wr1 Wr2 W2,
c_r = bl1 Wl2 W2, c_1 = (bl2 + bl1 Wr2) W2 + b2, r = (deg > 0).
The two aggregation passes therefore run on 3-feature vectors.

Distribution: NC n owns src-shard n (12500 nodes; computes h locally).
Its incident edges are grouped by dst-range -> Q7 core, chunked and sorted
by dst.  Per chunk: GPSIMD ap_gather (feature-per-partition tables, int16
src-local indices) -> DVE cumulative-sum scan -> GPSIMD gather at segment
ends -> shifted subtract = per-dst sums.  A PE one-hot matmul compacts the
(group, feature) partitions, and partial sums are ReduceScattered across
the 8 NCs (dst-shard n -> NC n), divided by degree on device, and fed to
pass 2.  Host-side work is only integer graph partitioning / index layout.
"""
import numpy as np

# ---- problem constants (hardcoded per contract) ----
N = 100000
E = 6400000
IN_F = 128
HID = 10
OUT = 3

NCN = 8             # NeuronCores
QC = 8              # Q7 cores per NC
NSH = N // NCN      # 12500 nodes per shard
K_CH = 8            # chunks per (NC, q7core)
D_CHUNK = -(-NSH // K_CH)          # 1042 dsts per chunk
D_STRIP = K_CH * D_CHUNK           # 12504
NE = ((D_CHUNK + 1 + 15) // 16) * 16   # 1056 ends entries per chunk
NSH_TAB = ((NSH + 16 + 15) // 16) * 16  # 12528 table width
SENT = NSH + 6                      # sentinel (zero) table column
XPAD = ((NSH + 511) // 512) * 512   # 12800 padded x rows (512-row groups)
F32 = "float32"

_prog_cache = {}


def _build_program(C_CAP):
    from contextlib import ExitStack
    import concourse.bacc as bacc
    import concourse.tile as tile
    import concourse.mybir as mybir
    from concourse.masks import make_identity

    f32 = mybir.dt.float32
    i16 = mybir.dt.int16
    AL = mybir.AluOpType
    AF = mybir.ActivationFunctionType

    nc = bacc.Bacc("TRN2", target_bir_lowering=False, debug=False,
                   num_devices=NCN)

    # ---- I/O ----
    x_in = nc.dram_tensor("x_sh", [XPAD, IN_F], f32, kind="ExternalInput")
    eidx_in = nc.dram_tensor("eidx", [K_CH, 128, C_CAP // 16], i16, kind="ExternalInput")
    eend_in = nc.dram_tensor("eends", [K_CH, 128, NE // 16], i16, kind="ExternalInput")
    deg_in = nc.dram_tensor("deg3", [3, D_STRIP], f32, kind="ExternalInput")
    sel_in = nc.dram_tensor("selAB", [128, 48], f32, kind="ExternalInput")
    rep72_in = nc.dram_tensor("rep72", [3, 72], f32, kind="ExternalInput")
    W1_in = nc.dram_tensor("W1", [IN_F, HID], f32, kind="ExternalInput")
    b1_in = nc.dram_tensor("b1c", [HID, 1], f32, kind="ExternalInput")
    Wl2_in = nc.dram_tensor("Wl2", [HID, HID], f32, kind="ExternalInput")
    Wr2_in = nc.dram_tensor("Wr2", [HID, HID], f32, kind="ExternalInput")
    W2_in = nc.dram_tensor("W2", [HID, OUT], f32, kind="ExternalInput")
    Wl1T_in = nc.dram_tensor("Wl1T", [HID, HID], f32, kind="ExternalInput")
    Wr1T_in = nc.dram_tensor("Wr1T", [HID, HID], f32, kind="ExternalInput")
    bl1_in = nc.dram_tensor("bl1c", [HID, 1], f32, kind="ExternalInput")
    bl2_in = nc.dram_tensor("bl2c", [HID, 1], f32, kind="ExternalInput")
    b2_in = nc.dram_tensor("b2c", [OUT, 1], f32, kind="ExternalInput")
    out_ext = nc.dram_tensor("outT", [3, D_STRIP], f32, kind="ExternalOutput")

    with tile.TileContext(nc) as tc:
        es = ExitStack()
        with es:
            dram = es.enter_context(tc.tile_pool(name="dram", bufs=1, space="DRAM"))
            p_small = es.enter_context(tc.tile_pool(name="small", bufs=1))

            gc0_d = dram.tile([3, D_STRIP], f32)
            WA = D_STRIP // 24          # 521: [3, D_STRIP] viewed as [72, 521]

            def v72(ap):
                return ap.rearrange("f (a b) -> (f a) b", a=24)
            bounceA = dram.tile([NCN, 3, D_STRIP], f32)
            bounceB = dram.tile([NCN, 3, D_STRIP], f32)
            bounceC = dram.tile([NCN, 3, D_STRIP], f32)
            rsA = dram.tile([3, D_STRIP], f32)
            rsB = dram.tile([3, D_STRIP], f32)
            rsC = dram.tile([3, D_STRIP], f32)

            sel = p_small.tile([128, 48], f32)
            nc.sync.dma_start(out=sel[:], in_=sel_in[:])

            # ---- phase 0: folded weight matrices (tiny PE matmuls) ----
            w1 = p_small.tile([IN_F, HID], f32)
            wl2 = p_small.tile([HID, HID], f32)
            wr2 = p_small.tile([HID, HID], f32)
            w2 = p_small.tile([HID, OUT], f32)
            wl1t = p_small.tile([HID, HID], f32)
            wr1t = p_small.tile([HID, HID], f32)
            b1c = p_small.tile([HID, 1], f32)
            bl1c = p_small.tile([HID, 1], f32)
            bl2c = p_small.tile([HID, 1], f32)
            b2c = p_small.tile([OUT, 1], f32)
            for t, src in [(w1, W1_in), (wl2, Wl2_in), (wr2, Wr2_in),
                           (w2, W2_in), (wl1t, Wl1T_in), (wr1t, Wr1T_in),
                           (b1c, b1_in), (bl1c, bl1_in), (bl2c, bl2_in),
                           (b2c, b2_in)]:
                nc.sync.dma_start(out=t[:], in_=src[:])

            rec_d = dram.tile([3, D_STRIP], f32)
            with tc.tile_pool(name="rec0", bufs=1) as p_rec:
                rt0 = p_rec.tile([72, WA], f32)
                nc.scalar.dma_start(out=rt0[:], in_=v72(deg_in[:]))
                nc.vector.tensor_scalar_max(out=rt0[:], in0=rt0[:], scalar1=1.0)
                nc.vector.reciprocal(out=rt0[:], in_=rt0[:])
                nc.scalar.dma_start(out=v72(rec_d[:]), in_=rt0[:])

            p_ps0 = es.enter_context(tc.tile_pool(name="psum0", bufs=1, space="PSUM"))

            def mm(lhsT, rhs, m, n_, accum=None):
                """matmul into fresh psum, copy to fresh small sbuf tile."""
                ps = p_ps0.tile([m, n_], f32, space="PSUM", tag="ps0")
                if accum is None:
                    nc.tensor.matmul(out=ps[:], lhsT=lhsT, rhs=rhs, start=True, stop=True)
                else:
                    nc.tensor.matmul(out=ps[:], lhsT=lhsT, rhs=rhs, start=True, stop=False)
                    nc.tensor.matmul(out=ps[:], lhsT=accum[0], rhs=accum[1], start=False, stop=True)
                sb = p_small.tile([m, n_], f32, tag=f"mm_{m}x{n_}_{nc.next_id()}")
                nc.vector.tensor_copy(out=sb[:], in_=ps[:])
                return sb

            s2 = mm(wl2[:], wl1t[:], HID, HID)                     # (Wl1 Wl2)^T
            rt = mm(wl2[:], wr1t[:], HID, HID, accum=(wr2[:], wl1t[:]))  # R^T
            s0 = mm(wr2[:], wr1t[:], HID, HID)                     # (Wr1 Wr2)^T
            ccc = p_small.tile([HID, 9], f32)
            for j, lh in [(0, rt), (3, s2), (6, s0)]:
                ps = p_ps0.tile([HID, OUT], f32, space="PSUM", tag="ps0")
                nc.tensor.matmul(out=ps[:], lhsT=lh[:], rhs=w2[:], start=True, stop=True)
                nc.vector.tensor_copy(out=ccc[:, j:j + 3], in_=ps[:])
            t1 = mm(wl2[:], bl1c[:], HID, 1)
            crs = mm(w2[:], t1[:], OUT, 1)                          # c_r [3,1]
            u = mm(wr2[:], bl1c[:], HID, 1)
            nc.vector.tensor_tensor(out=u[:], in0=u[:], in1=bl2c[:], op=AL.add)
            c1s = mm(w2[:], u[:], OUT, 1)                           # pre b2
            nc.vector.tensor_tensor(out=c1s[:], in0=c1s[:], in1=b2c[:], op=AL.add)

            # ---- phase 1: h = relu(x W1 + b1); gc = [hC1 | hC2 | hC0] ----
            p_tab = es.enter_context(tc.tile_pool(name="tab", bufs=1))
            tab = p_tab.tile([128, NSH_TAB], f32)
            nc.scalar.memzero(tab[:])
            with tc.tile_pool(name="lin1", bufs=4) as p_lin, \
                 tc.tile_pool(name="lin1gc", bufs=1) as p_gc, \
                 tc.tile_pool(name="lin1ps", bufs=2, space="PSUM") as p_lps:
                ident = p_small.tile([128, 128], f32)
                make_identity(nc, ident[:])
                gcf = p_gc.tile([9, XPAD], f32)
                n_grp = XPAD // 512
                for g in range(n_grp):
                    xt4 = p_lin.tile([128, 4, 128], f32, tag="xt4")
                    # rows 512g..512g+512 of x -> [p, t, f]
                    nc.sync.dma_start(
                        out=xt4[:],
                        in_=x_in[:].rearrange("(a t p) f -> a p t f", a=XPAD // 512, t=4, p=128)[g])
                    tps = p_lps.tile([128, 512], f32, space="PSUM", tag="tps")
                    for t in range(4):
                        nc.tensor.transpose(out=tps[:, t * 128:(t + 1) * 128],
                                            in_=xt4[:, t, :], identity=ident[:])
                    xtb = p_lin.tile([128, 512], f32, tag="xtb")
                    nc.vector.tensor_copy(out=xtb[:], in_=tps[:])
                    hps = p_lps.tile([HID, 512], f32, space="PSUM", tag="hps")
                    nc.tensor.matmul(out=hps[:], lhsT=w1[:], rhs=xtb[:], start=True, stop=True)
                    hb = p_lin.tile([HID, 512], f32, tag="hb")
                    nc.scalar.activation(out=hb[:], in_=hps[:], func=AF.Relu,
                                         bias=b1c[:], scale=1.0)
                    gps = p_lps.tile([9, 512], f32, space="PSUM", tag="gps")
                    nc.tensor.matmul(out=gps[:], lhsT=ccc[:], rhs=hb[:], start=True, stop=True)
                    nc.vector.tensor_copy(out=gcf[:, g * 512:(g + 1) * 512], in_=gps[:])
                # distribute into gather table (per 16-partition group) + gc0 out
                tengs = [nc.sync, nc.scalar, nc.gpsimd]
                for g in range(QC):
                    tengs[g % 3].dma_start(out=tab[16 * g:16 * g + 6, 0:NSH],
                                           in_=gcf[0:6, 0:NSH])
                nc.scalar.dma_start(out=gc0_d[:], in_=gcf[6:9, 0:D_STRIP])

            # ---- aggregation passes (software-pipelined on the GPSIMD queue:
            # chunk k+1's main gather is issued before chunk k's ends gather
            # so the Q7 never idles waiting on the DVE scan) ----
            def agg_pass(bounces):
                """bounces: list of (comp_row_offset, dram_view24) to store."""
                with tc.tile_pool(name="agg_msg", bufs=2) as p_msg, \
                     tc.tile_pool(name="agg_sm", bufs=2) as p_asm, \
                     tc.tile_pool(name="agg_ps", bufs=2, space="PSUM") as p_aps:
                    live = {}

                    def front(k):
                        idx_t = p_asm.tile([128, C_CAP // 16], i16, tag="idx")
                        nc.sync.dma_start(out=idx_t[:], in_=eidx_in[k])
                        end_t = p_asm.tile([128, NE // 16], i16, tag="end")
                        nc.sync.dma_start(out=end_t[:], in_=eend_in[k])
                        msg = p_msg.tile([128, C_CAP], f32, tag="msg")
                        nc.gpsimd.ap_gather(
                            out_ap=msg[:], in_ap=tab[:], idxs_ap=idx_t[:],
                            channels=128, num_elems=NSH_TAB, d=1, num_idxs=C_CAP)
                        nc.vector.tensor_tensor_scan(
                            out=msg[:], data0=msg[:], data1=msg[:], initial=0.0,
                            op0=AL.add, op1=AL.bypass)
                        live[k] = (msg, end_t)

                    def back(k):
                        msg, end_t = live.pop(k)
                        gat = p_asm.tile([128, NE], f32, tag="gat")
                        nc.gpsimd.ap_gather(
                            out_ap=gat[:], in_ap=msg[:], idxs_ap=end_t[:],
                            channels=128, num_elems=C_CAP, d=1, num_idxs=NE)
                        strip = p_asm.tile([128, D_CHUNK], f32, tag="strip")
                        nc.vector.tensor_tensor(
                            out=strip[:], in0=gat[:, 1:1 + D_CHUNK],
                            in1=gat[:, 0:D_CHUNK], op=AL.subtract)
                        comp = p_asm.tile([48, D_CHUNK], f32, tag="comp")
                        for j in range(0, D_CHUNK, 512):
                            w = min(512, D_CHUNK - j)
                            cps = p_aps.tile([48, w], f32, space="PSUM", tag="cps")
                            nc.tensor.matmul(out=cps[:], lhsT=sel[:],
                                             rhs=strip[:, j:j + w], start=True, stop=True)
                            nc.vector.tensor_copy(out=comp[:, j:j + w], in_=cps[:])
                        for off, view24 in bounces:
                            nc.sync.dma_start(
                                out=view24[:, k * D_CHUNK:(k + 1) * D_CHUNK],
                                in_=comp[off:off + 24, :])

                    for k in range(K_CH):
                        front(k)
                        if k >= 1:
                            back(k - 1)
                    back(K_CH - 1)

            vA = bounceA[:].rearrange("g f d -> (g f) d")
            vB = bounceB[:].rearrange("g f d -> (g f) d")
            vC = bounceC[:].rearrange("g f d -> (g f) d")

            agg_pass([(0, vA), (24, vB)])

            rg = [list(range(NCN))]
            nc.gpsimd.collective_compute("ReduceScatter", AL.add, replica_groups=rg,
                                         ins=[bounceA.opt()], outs=[rsA.opt()])

            # build pass-2 table: a2' = rsA * recip, replicated per group
            nc.scalar.memzero(tab[:])
            with tc.tile_pool(name="mid", bufs=2) as p_mid:
                ta = p_mid.tile([3, D_STRIP], f32, tag="wide")
                nc.sync.dma_start(out=ta[:], in_=rsA[:])
                td = p_mid.tile([3, D_STRIP], f32, tag="wide")
                nc.scalar.dma_start(out=td[:], in_=rec_d[:])
                nc.vector.tensor_tensor(out=ta[:], in0=ta[:], in1=td[:], op=AL.mult)
                for g in range(QC):
                    eng = nc.sync if g % 2 == 0 else nc.scalar
                    eng.dma_start(out=tab[16 * g:16 * g + 3, 0:D_STRIP], in_=ta[:])

            # RS of the B partials overlaps pass 2 (collectives are async
            # w.r.t. the issuing queue; rsB is only read by final assembly)
            nc.gpsimd.collective_compute("ReduceScatter", AL.add, replica_groups=rg,
                                         ins=[bounceB.opt()], outs=[rsB.opt()])

            agg_pass([(24, vC)])

            nc.gpsimd.collective_compute("ReduceScatter", AL.add, replica_groups=rg,
                                         ins=[bounceC.opt()], outs=[rsC.opt()])

            # replicate [crs | c1s] to 72 partitions for the [72, WA] final math
            rep72 = p_small.tile([3, 72], f32)
            nc.sync.dma_start(out=rep72[:], in_=rep72_in[:])
            cc2 = p_small.tile([3, 2], f32)
            nc.vector.tensor_copy(out=cc2[:, 0:1], in_=crs[:])
            nc.vector.tensor_copy(out=cc2[:, 1:2], in_=c1s[:])
            ps72 = p_ps0.tile([72, 2], f32, space="PSUM", tag="ps0")
            nc.tensor.matmul(out=ps72[:], lhsT=rep72[:], rhs=cc2[:], start=True, stop=True)
            crsc = p_small.tile([72, 2], f32)
            nc.vector.tensor_copy(out=crsc[:], in_=ps72[:])
            # ---- final assembly: out = (a1+b)*recip + gc0 + r*c_r + c_1 ----
            # all elementwise math on a [72, WA] view (24x the lanes of [3, D_STRIP])
            with tc.tile_pool(name="fin", bufs=3) as p_fin:
                s1 = p_fin.tile([72, WA], f32, tag="fw")
                nc.sync.dma_start(out=s1[:], in_=v72(rsB[:]))
                s2_ = p_fin.tile([72, WA], f32, tag="fw")
                nc.sync.dma_start(out=s2_[:], in_=v72(rsC[:]))
                nc.vector.tensor_tensor(out=s1[:], in0=s1[:], in1=s2_[:], op=AL.add)
                sd = p_fin.tile([72, WA], f32, tag="fw")
                nc.scalar.dma_start(out=sd[:], in_=v72(deg_in[:]))
                sr = p_fin.tile([72, WA], f32, tag="fw")
                nc.sync.dma_start(out=sr[:], in_=v72(rec_d[:]))
                nc.vector.tensor_tensor(out=s1[:], in0=s1[:], in1=sr[:], op=AL.mult)
                # r = (deg > 0); s1 += r * c_r
                nc.vector.tensor_scalar(out=sd[:], in0=sd[:], scalar1=0.0, scalar2=None,
                                        op0=AL.is_gt)
                nc.vector.scalar_tensor_tensor(out=s1[:], in0=sd[:], scalar=crsc[:, 0:1],
                                               in1=s1[:], op0=AL.mult, op1=AL.add)
                sg = p_fin.tile([72, WA], f32, tag="fw")
                nc.sync.dma_start(out=sg[:], in_=v72(gc0_d[:]))
                nc.vector.tensor_tensor(out=s1[:], in0=s1[:], in1=sg[:], op=AL.add)
                nc.scalar.activation(out=s1[:], in_=s1[:], func=AF.Identity,
                                     bias=crsc[:, 1:2], scale=1.0)
                nc.sync.dma_start(out=v72(out_ext[:]), in_=s1[:])

    nc.compile()
    return nc


def _preprocess(inputs):
    """Pure-integer graph partitioning + layout; returns per-core in_maps."""
    x = np.ascontiguousarray(np.asarray(inputs["x"], dtype=np.float32))
    edge_index = np.asarray(inputs["edge_index"])
    src = edge_index[0].astype(np.int64)
    dst = edge_index[1].astype(np.int64)

    n_of_e = src // NSH
    c_of_e = dst // NSH
    k_of_e = (dst % NSH) // D_CHUNK
    d_of_e = (dst % NSH) % D_CHUNK
    src_loc = (src % NSH).astype(np.int16)

    key = ((n_of_e * QC + c_of_e) * K_CH + k_of_e) * np.int64(N) + dst
    order = np.argsort(key, kind="stable")
    so = src_loc[order]
    no = n_of_e[order]
    co = c_of_e[order]
    ko = k_of_e[order]
    do_ = d_of_e[order]

    cell = (no * QC + co) * K_CH + ko
    n_cells = NCN * QC * K_CH
    cnt = np.bincount(cell, minlength=n_cells)
    C_CAP = ((int(cnt.max()) + 1 + 15) // 16) * 16
    cell_start = np.zeros(n_cells + 1, np.int64)
    np.cumsum(cnt, out=cell_start[1:])
    slot = np.arange(E) - cell_start[cell] + 1        # slot 0 = sentinel pad

    eidx = np.full((NCN, K_CH, 128, C_CAP // 16), SENT, np.int16)
    eidx[no, ko, 16 * co + (slot % 16), slot // 16] = so

    dcell = cell * D_CHUNK + do_
    cnt_d = np.bincount(dcell, minlength=n_cells * D_CHUNK).reshape(
        NCN, QC, K_CH, D_CHUNK)
    ce = np.cumsum(cnt_d, axis=3).astype(np.int16)
    ends_ext = np.zeros((NCN, QC, K_CH, NE), np.int16)
    ends_ext[:, :, :, 1:D_CHUNK + 1] = ce
    ends_ext[:, :, :, D_CHUNK + 1:] = ce[:, :, :, -1:]
    eends = np.zeros((NCN, K_CH, 128, NE // 16), np.int16)
    for c in range(QC):
        w = ends_ext[:, c].reshape(NCN, K_CH, NE // 16, 16)
        eends[:, :, 16 * c:16 * c + 16, :] = np.swapaxes(w, 2, 3)

    deg = np.bincount(dst, minlength=N).astype(np.float32)
    deg3 = np.zeros((NCN, 3, D_STRIP), np.float32)
    for n in range(NCN):
        deg3[n, :, :NSH] = deg[n * NSH:(n + 1) * NSH][None, :]

    sel = np.zeros((128, 48), np.float32)
    for g in range(QC):
        for f in range(3):
            sel[16 * g + 3 + f, 3 * g + f] = 1.0       # A set: C2/a2 rows
            sel[16 * g + f, 24 + 3 * g + f] = 1.0      # B set: C1/a1 rows

    rep72 = np.zeros((3, 72), np.float32)
    for f in range(3):
        rep72[f, 24 * f:24 * (f + 1)] = 1.0

    W1 = np.ascontiguousarray(np.asarray(inputs["W1"], np.float32))
    wmap = {
        "W1": W1,
        "rep72": rep72,
        "b1c": np.ascontiguousarray(np.asarray(inputs["b1"], np.float32).reshape(HID, 1)),
        "Wl2": np.ascontiguousarray(np.asarray(inputs["Wl2"], np.float32)),
        "Wr2": np.ascontiguousarray(np.asarray(inputs["Wr2"], np.float32)),
        "W2": np.ascontiguousarray(np.asarray(inputs["W2"], np.float32)),
        "Wl1T": np.ascontiguousarray(np.asarray(inputs["Wl1"], np.float32).T),
        "Wr1T": np.ascontiguousarray(np.asarray(inputs["Wr1"], np.float32).T),
        "bl1c": np.ascontiguousarray(np.asarray(inputs["bl1"], np.float32).reshape(HID, 1)),
        "bl2c": np.ascontiguousarray(np.asarray(inputs["bl2"], np.float32).reshape(HID, 1)),
        "b2c": np.ascontiguousarray(np.asarray(inputs["b2"], np.float32).reshape(OUT, 1)),
    }

    in_maps = []
    for n in range(NCN):
        xs = np.zeros((XPAD, IN_F), np.float32)
        xs[:NSH] = x[n * NSH:(n + 1) * NSH]
        m = {
            "x_sh": xs,
            "eidx": np.ascontiguousarray(eidx[n]),
            "eends": np.ascontiguousarray(eends[n]),
            "deg3": np.ascontiguousarray(deg3[n]),
            "selAB": sel,
        }
        m.update(wmap)
        in_maps.append(m)
    return in_maps, C_CAP


def kernel(**inputs) -> np.ndarray:
    from concourse.bass_utils import run_bass_kernel_spmd

    in_maps, C_CAP = _preprocess(inputs)
    if C_CAP not in _prog_cache:
        _prog_cache[C_CAP] = _build_program(C_CAP)
    nc = _prog_cache[C_CAP]
    res = run_bass_kernel_spmd(nc, in_maps, core_ids=list(range(NCN)))
    out = np.empty((N, OUT), np.float32)
    for n in range(NCN):
        out[n * NSH:(n + 1) * NSH] = res.results[n]["outT"].T[:NSH]
    return out



# revision 22
# speedup vs baseline: 1.0023x; 1.0012x over previous
"""Trainium2 Bass kernel for 2-layer GraphSAGE (mean aggregation) on 8 NeuronCores.

Math: with M = mean-aggregation operator (D^-1 A), the reference is
    h  = relu(x @ W1 + b1)
    h1 = (M h) Wl1 + bl1 + h Wr1
    h2 = (M h1) Wl2 + bl2 + h1 Wr2
    out = h2 @ W2 + b2
Everything after the relu is linear, so fold:
    out = M(M(h C2)) + M(h C1) + h C0 + r*c_r + c_1
with C2 = Wl1 Wl2 W2, C1 = (Wr1 Wl2 + Wl1 Wr2) W2, C0 = Wr1 Wr2 W2,
c_r = bl1 Wl2 W2, c_1 = (bl2 + bl1 Wr2) W2 + b2, r = (deg > 0).
The two aggregation passes therefore run on 3-feature vectors.

Distribution: NC n owns src-shard n (12500 nodes; computes h locally).
Its incident edges are grouped by dst-range -> Q7 core, chunked and sorted
by dst.  Per chunk: GPSIMD ap_gather (feature-per-partition tables, int16
src-local indices) -> DVE cumulative-sum scan -> GPSIMD gather at segment
ends -> shifted subtract = per-dst sums.  A PE one-hot matmul compacts the
(group, feature) partitions, and partial sums are ReduceScattered across
the 8 NCs (dst-shard n -> NC n), divided by degree on device, and fed to
pass 2.  Host-side work is only integer graph partitioning / index layout.
"""
import numpy as np

# ---- problem constants (hardcoded per contract) ----
N = 100000
E = 6400000
IN_F = 128
HID = 10
OUT = 3

NCN = 8             # NeuronCores
QC = 8              # Q7 cores per NC
NSH = N // NCN      # 12500 nodes per shard
K_CH = 8            # chunks per (NC, q7core)
D_CHUNK = -(-NSH // K_CH)          # 1042 dsts per chunk
D_STRIP = K_CH * D_CHUNK           # 12504
NE = ((D_CHUNK + 1 + 15) // 16) * 16   # 1056 ends entries per chunk
NSH_TAB = ((NSH + 16 + 15) // 16) * 16  # 12528 table width
SENT = NSH + 6                      # sentinel (zero) table column
XPAD = ((NSH + 511) // 512) * 512   # 12800 padded x rows (512-row groups)
F32 = "float32"

_prog_cache = {}


def _build_program(C_CAP):
    from contextlib import ExitStack
    import concourse.bacc as bacc
    import concourse.tile as tile
    import concourse.mybir as mybir
    from concourse.masks import make_identity

    f32 = mybir.dt.float32
    i16 = mybir.dt.int16
    AL = mybir.AluOpType
    AF = mybir.ActivationFunctionType

    nc = bacc.Bacc("TRN2", target_bir_lowering=False, debug=False,
                   num_devices=NCN)

    # ---- I/O ----
    x_in = nc.dram_tensor("x_sh", [XPAD, IN_F], f32, kind="ExternalInput")
    eidx_in = nc.dram_tensor("eidx", [K_CH, 128, C_CAP // 16], i16, kind="ExternalInput")
    eend_in = nc.dram_tensor("eends", [K_CH, 128, NE // 16], i16, kind="ExternalInput")
    deg_in = nc.dram_tensor("deg3", [3, D_STRIP], f32, kind="ExternalInput")
    sel_in = nc.dram_tensor("selAB", [128, 48], f32, kind="ExternalInput")
    rep72_in = nc.dram_tensor("rep72", [3, 72], f32, kind="ExternalInput")
    W1_in = nc.dram_tensor("W1", [IN_F, HID], f32, kind="ExternalInput")
    b1_in = nc.dram_tensor("b1c", [HID, 1], f32, kind="ExternalInput")
    Wl2_in = nc.dram_tensor("Wl2", [HID, HID], f32, kind="ExternalInput")
    Wr2_in = nc.dram_tensor("Wr2", [HID, HID], f32, kind="ExternalInput")
    W2_in = nc.dram_tensor("W2", [HID, OUT], f32, kind="ExternalInput")
    Wl1T_in = nc.dram_tensor("Wl1T", [HID, HID], f32, kind="ExternalInput")
    Wr1T_in = nc.dram_tensor("Wr1T", [HID, HID], f32, kind="ExternalInput")
    bl1_in = nc.dram_tensor("bl1c", [HID, 1], f32, kind="ExternalInput")
    bl2_in = nc.dram_tensor("bl2c", [HID, 1], f32, kind="ExternalInput")
    b2_in = nc.dram_tensor("b2c", [OUT, 1], f32, kind="ExternalInput")
    out_ext = nc.dram_tensor("outT", [3, D_STRIP], f32, kind="ExternalOutput")

    with tile.TileContext(nc) as tc:
        es = ExitStack()
        with es:
            dram = es.enter_context(tc.tile_pool(name="dram", bufs=1, space="DRAM"))
            p_small = es.enter_context(tc.tile_pool(name="small", bufs=1))

            gc0_d = dram.tile([3, D_STRIP], f32)
            WA = D_STRIP // 24          # 521: [3, D_STRIP] viewed as [72, 521]

            def v72(ap):
                return ap.rearrange("f (a b) -> (f a) b", a=24)
            bounceA = dram.tile([NCN, 3, D_STRIP], f32)
            bounceB = dram.tile([NCN, 3, D_STRIP], f32)
            bounceC = dram.tile([NCN, 3, D_STRIP], f32)
            rsA = dram.tile([3, D_STRIP], f32)
            rsB = dram.tile([3, D_STRIP], f32)
            rsC = dram.tile([3, D_STRIP], f32)

            sel = p_small.tile([128, 48], f32)
            nc.sync.dma_start(out=sel[:], in_=sel_in[:])

            # ---- phase 0: folded weight matrices (tiny PE matmuls) ----
            w1 = p_small.tile([IN_F, HID], f32)
            wl2 = p_small.tile([HID, HID], f32)
            wr2 = p_small.tile([HID, HID], f32)
            w2 = p_small.tile([HID, OUT], f32)
            wl1t = p_small.tile([HID, HID], f32)
            wr1t = p_small.tile([HID, HID], f32)
            b1c = p_small.tile([HID, 1], f32)
            bl1c = p_small.tile([HID, 1], f32)
            bl2c = p_small.tile([HID, 1], f32)
            b2c = p_small.tile([OUT, 1], f32)
            for t, src in [(w1, W1_in), (wl2, Wl2_in), (wr2, Wr2_in),
                           (w2, W2_in), (wl1t, Wl1T_in), (wr1t, Wr1T_in),
                           (b1c, b1_in), (bl1c, bl1_in), (bl2c, bl2_in),
                           (b2c, b2_in)]:
                nc.sync.dma_start(out=t[:], in_=src[:])

            rec_d = dram.tile([3, D_STRIP], f32)
            with tc.tile_pool(name="rec0", bufs=1) as p_rec:
                rt0 = p_rec.tile([72, WA], f32)
                nc.scalar.dma_start(out=rt0[:], in_=v72(deg_in[:]))
                nc.vector.tensor_scalar_max(out=rt0[:], in0=rt0[:], scalar1=1.0)
                nc.vector.reciprocal(out=rt0[:], in_=rt0[:])
                nc.scalar.dma_start(out=v72(rec_d[:]), in_=rt0[:])

            p_ps0 = es.enter_context(tc.tile_pool(name="psum0", bufs=1, space="PSUM"))

            def mm(lhsT, rhs, m, n_, accum=None):
                """matmul into fresh psum, copy to fresh small sbuf tile."""
                ps = p_ps0.tile([m, n_], f32, space="PSUM", tag="ps0")
                if accum is None:
                    nc.tensor.matmul(out=ps[:], lhsT=lhsT, rhs=rhs, start=True, stop=True)
                else:
                    nc.tensor.matmul(out=ps[:], lhsT=lhsT, rhs=rhs, start=True, stop=False)
                    nc.tensor.matmul(out=ps[:], lhsT=accum[0], rhs=accum[1], start=False, stop=True)
                sb = p_small.tile([m, n_], f32, tag=f"mm_{m}x{n_}_{nc.next_id()}")
                nc.vector.tensor_copy(out=sb[:], in_=ps[:])
                return sb

            s2 = mm(wl2[:], wl1t[:], HID, HID)                     # (Wl1 Wl2)^T
            rt = mm(wl2[:], wr1t[:], HID, HID, accum=(wr2[:], wl1t[:]))  # R^T
            s0 = mm(wr2[:], wr1t[:], HID, HID)                     # (Wr1 Wr2)^T
            ccc = p_small.tile([HID, 9], f32)
            for j, lh in [(0, rt), (3, s2), (6, s0)]:
                ps = p_ps0.tile([HID, OUT], f32, space="PSUM", tag="ps0")
                nc.tensor.matmul(out=ps[:], lhsT=lh[:], rhs=w2[:], start=True, stop=True)
                nc.vector.tensor_copy(out=ccc[:, j:j + 3], in_=ps[:])
            t1 = mm(wl2[:], bl1c[:], HID, 1)
            crs = mm(w2[:], t1[:], OUT, 1)                          # c_r [3,1]
            u = mm(wr2[:], bl1c[:], HID, 1)
            nc.vector.tensor_tensor(out=u[:], in0=u[:], in1=bl2c[:], op=AL.add)
            c1s = mm(w2[:], u[:], OUT, 1)                           # pre b2
            nc.vector.tensor_tensor(out=c1s[:], in0=c1s[:], in1=b2c[:], op=AL.add)

            # ---- phase 1: h = relu(x W1 + b1); gc = [hC1 | hC2 | hC0] ----
            p_tab = es.enter_context(tc.tile_pool(name="tab", bufs=1))
            tab = p_tab.tile([128, NSH_TAB], f32)
            nc.scalar.memzero(tab[:])
            with tc.tile_pool(name="lin1", bufs=2) as p_lin, \
                 tc.tile_pool(name="lin1gc", bufs=1) as p_gc, \
                 tc.tile_pool(name="lin1ps", bufs=2, space="PSUM") as p_lps:
                ident = p_small.tile([128, 128], f32)
                make_identity(nc, ident[:])
                gcf = p_gc.tile([9, XPAD], f32)
                n_grp = XPAD // 512
                for g in range(n_grp):
                    xt4 = p_lin.tile([128, 4, 128], f32, tag="xt4")
                    # rows 512g..512g+512 of x -> [p, t, f]
                    nc.sync.dma_start(
                        out=xt4[:],
                        in_=x_in[:].rearrange("(a t p) f -> a p t f", a=XPAD // 512, t=4, p=128)[g])
                    tps = p_lps.tile([128, 512], f32, space="PSUM", tag="tps")
                    for t in range(4):
                        nc.tensor.transpose(out=tps[:, t * 128:(t + 1) * 128],
                                            in_=xt4[:, t, :], identity=ident[:])
                    xtb = p_lin.tile([128, 512], f32, tag="xtb")
                    nc.vector.tensor_copy(out=xtb[:], in_=tps[:])
                    hps = p_lps.tile([HID, 512], f32, space="PSUM", tag="hps")
                    nc.tensor.matmul(out=hps[:], lhsT=w1[:], rhs=xtb[:], start=True, stop=True)
                    hb = p_lin.tile([HID, 512], f32, tag="hb")
                    nc.scalar.activation(out=hb[:], in_=hps[:], func=AF.Relu,
                                         bias=b1c[:], scale=1.0)
                    gps = p_lps.tile([9, 512], f32, space="PSUM", tag="gps")
                    nc.tensor.matmul(out=gps[:], lhsT=ccc[:], rhs=hb[:], start=True, stop=True)
                    nc.vector.tensor_copy(out=gcf[:, g * 512:(g + 1) * 512], in_=gps[:])
                # distribute into gather table (per 16-partition group) + gc0 out
                for g in range(QC):
                    eng = nc.sync if g % 2 == 0 else nc.scalar
                    eng.dma_start(out=tab[16 * g:16 * g + 6, 0:NSH],
                                  in_=gcf[0:6, 0:NSH])
                nc.sync.dma_start(out=gc0_d[:], in_=gcf[6:9, 0:D_STRIP])

            # ---- aggregation passes (software-pipelined on the GPSIMD queue:
            # chunk k+1's main gather is issued before chunk k's ends gather
            # so the Q7 never idles waiting on the DVE scan) ----
            def agg_pass(bounces):
                """bounces: list of (comp_row_offset, dram_view24) to store."""
                with tc.tile_pool(name="agg_msg", bufs=2) as p_msg, \
                     tc.tile_pool(name="agg_sm", bufs=2) as p_asm, \
                     tc.tile_pool(name="agg_ps", bufs=2, space="PSUM") as p_aps:
                    live = {}

                    def front(k):
                        idx_t = p_asm.tile([128, C_CAP // 16], i16, tag="idx")
                        nc.sync.dma_start(out=idx_t[:], in_=eidx_in[k])
                        end_t = p_asm.tile([128, NE // 16], i16, tag="end")
                        nc.sync.dma_start(out=end_t[:], in_=eend_in[k])
                        msg = p_msg.tile([128, C_CAP], f32, tag="msg")
                        nc.gpsimd.ap_gather(
                            out_ap=msg[:], in_ap=tab[:], idxs_ap=idx_t[:],
                            channels=128, num_elems=NSH_TAB, d=1, num_idxs=C_CAP)
                        nc.vector.tensor_tensor_scan(
                            out=msg[:], data0=msg[:], data1=msg[:], initial=0.0,
                            op0=AL.add, op1=AL.bypass)
                        live[k] = (msg, end_t)

                    def back(k):
                        msg, end_t = live.pop(k)
                        gat = p_asm.tile([128, NE], f32, tag="gat")
                        nc.gpsimd.ap_gather(
                            out_ap=gat[:], in_ap=msg[:], idxs_ap=end_t[:],
                            channels=128, num_elems=C_CAP, d=1, num_idxs=NE)
                        strip = p_asm.tile([128, D_CHUNK], f32, tag="strip")
                        nc.vector.tensor_tensor(
                            out=strip[:], in0=gat[:, 1:1 + D_CHUNK],
                            in1=gat[:, 0:D_CHUNK], op=AL.subtract)
                        comp = p_asm.tile([48, D_CHUNK], f32, tag="comp")
                        for j in range(0, D_CHUNK, 512):
                            w = min(512, D_CHUNK - j)
                            cps = p_aps.tile([48, w], f32, space="PSUM", tag="cps")
                            nc.tensor.matmul(out=cps[:], lhsT=sel[:],
                                             rhs=strip[:, j:j + w], start=True, stop=True)
                            nc.vector.tensor_copy(out=comp[:, j:j + w], in_=cps[:])
                        for off, view24 in bounces:
                            nc.sync.dma_start(
                                out=view24[:, k * D_CHUNK:(k + 1) * D_CHUNK],
                                in_=comp[off:off + 24, :])

                    for k in range(K_CH):
                        front(k)
                        if k >= 1:
                            back(k - 1)
                    back(K_CH - 1)

            vA = bounceA[:].rearrange("g f d -> (g f) d")
            vB = bounceB[:].rearrange("g f d -> (g f) d")
            vC = bounceC[:].rearrange("g f d -> (g f) d")

            agg_pass([(0, vA), (24, vB)])

            rg = [list(range(NCN))]
            nc.gpsimd.collective_compute("ReduceScatter", AL.add, replica_groups=rg,
                                         ins=[bounceA.opt()], outs=[rsA.opt()])

            # build pass-2 table: a2' = rsA * recip, replicated per group
            nc.scalar.memzero(tab[:])
            with tc.tile_pool(name="mid", bufs=2) as p_mid:
                ta = p_mid.tile([3, D_STRIP], f32, tag="wide")
                nc.sync.dma_start(out=ta[:], in_=rsA[:])
                td = p_mid.tile([3, D_STRIP], f32, tag="wide")
                nc.scalar.dma_start(out=td[:], in_=rec_d[:])
                nc.vector.tensor_tensor(out=ta[:], in0=ta[:], in1=td[:], op=AL.mult)
                for g in range(QC):
                    eng = nc.sync if g % 2 == 0 else nc.scalar
                    eng.dma_start(out=tab[16 * g:16 * g + 3, 0:D_STRIP], in_=ta[:])

            # RS of the B partials overlaps pass 2 (collectives are async
            # w.r.t. the issuing queue; rsB is only read by final assembly)
            nc.gpsimd.collective_compute("ReduceScatter", AL.add, replica_groups=rg,
                                         ins=[bounceB.opt()], outs=[rsB.opt()])

            agg_pass([(24, vC)])

            nc.gpsimd.collective_compute("ReduceScatter", AL.add, replica_groups=rg,
                                         ins=[bounceC.opt()], outs=[rsC.opt()])

            # replicate [crs | c1s] to 72 partitions for the [72, WA] final math
            rep72 = p_small.tile([3, 72], f32)
            nc.sync.dma_start(out=rep72[:], in_=rep72_in[:])
            cc2 = p_small.tile([3, 2], f32)
            nc.vector.tensor_copy(out=cc2[:, 0:1], in_=crs[:])
            nc.vector.tensor_copy(out=cc2[:, 1:2], in_=c1s[:])
            ps72 = p_ps0.tile([72, 2], f32, space="PSUM", tag="ps0")
            nc.tensor.matmul(out=ps72[:], lhsT=rep72[:], rhs=cc2[:], start=True, stop=True)
            crsc = p_small.tile([72, 2], f32)
            nc.vector.tensor_copy(out=crsc[:], in_=ps72[:])
            # ---- final assembly: out = (a1+b)*recip + gc0 + r*c_r + c_1 ----
            # all elementwise math on a [72, WA] view (24x the lanes of [3, D_STRIP])
            with tc.tile_pool(name="fin", bufs=3) as p_fin:
                s1 = p_fin.tile([72, WA], f32, tag="fw")
                nc.sync.dma_start(out=s1[:], in_=v72(rsB[:]))
                s2_ = p_fin.tile([72, WA], f32, tag="fw")
                nc.sync.dma_start(out=s2_[:], in_=v72(rsC[:]))
                nc.vector.tensor_tensor(out=s1[:], in0=s1[:], in1=s2_[:], op=AL.add)
                sd = p_fin.tile([72, WA], f32, tag="fw")
                nc.scalar.dma_start(out=sd[:], in_=v72(deg_in[:]))
                sr = p_fin.tile([72, WA], f32, tag="fw")
                nc.sync.dma_start(out=sr[:], in_=v72(rec_d[:]))
                nc.vector.tensor_tensor(out=s1[:], in0=s1[:], in1=sr[:], op=AL.mult)
                # r = (deg > 0); s1 += r * c_r
                nc.vector.tensor_scalar(out=sd[:], in0=sd[:], scalar1=0.0, scalar2=None,
                                        op0=AL.is_gt)
                nc.vector.scalar_tensor_tensor(out=s1[:], in0=sd[:], scalar=crsc[:, 0:1],
                                               in1=s1[:], op0=AL.mult, op1=AL.add)
                sg = p_fin.tile([72, WA], f32, tag="fw")
                nc.sync.dma_start(out=sg[:], in_=v72(gc0_d[:]))
                nc.vector.tensor_tensor(out=s1[:], in0=s1[:], in1=sg[:], op=AL.add)
                nc.scalar.activation(out=s1[:], in_=s1[:], func=AF.Identity,
                                     bias=crsc[:, 1:2], scale=1.0)
                nc.sync.dma_start(out=v72(out_ext[:]), in_=s1[:])

    nc.compile()
    return nc


def _preprocess(inputs):
    """Pure-integer graph partitioning + layout; returns per-core in_maps."""
    x = np.ascontiguousarray(np.asarray(inputs["x"], dtype=np.float32))
    edge_index = np.asarray(inputs["edge_index"])
    src = edge_index[0].astype(np.int64)
    dst = edge_index[1].astype(np.int64)

    n_of_e = src // NSH
    c_of_e = dst // NSH
    k_of_e = (dst % NSH) // D_CHUNK
    d_of_e = (dst % NSH) % D_CHUNK
    src_loc = (src % NSH).astype(np.int16)

    key = ((n_of_e * QC + c_of_e) * K_CH + k_of_e) * np.int64(N) + dst
    order = np.argsort(key, kind="stable")
    so = src_loc[order]
    no = n_of_e[order]
    co = c_of_e[order]
    ko = k_of_e[order]
    do_ = d_of_e[order]

    cell = (no * QC + co) * K_CH + ko
    n_cells = NCN * QC * K_CH
    cnt = np.bincount(cell, minlength=n_cells)
    C_CAP = ((int(cnt.max()) + 1 + 15) // 16) * 16
    cell_start = np.zeros(n_cells + 1, np.int64)
    np.cumsum(cnt, out=cell_start[1:])
    slot = np.arange(E) - cell_start[cell] + 1        # slot 0 = sentinel pad

    eidx = np.full((NCN, K_CH, 128, C_CAP // 16), SENT, np.int16)
    eidx[no, ko, 16 * co + (slot % 16), slot // 16] = so

    dcell = cell * D_CHUNK + do_
    cnt_d = np.bincount(dcell, minlength=n_cells * D_CHUNK).reshape(
        NCN, QC, K_CH, D_CHUNK)
    ce = np.cumsum(cnt_d, axis=3).astype(np.int16)
    ends_ext = np.zeros((NCN, QC, K_CH, NE), np.int16)
    ends_ext[:, :, :, 1:D_CHUNK + 1] = ce
    ends_ext[:, :, :, D_CHUNK + 1:] = ce[:, :, :, -1:]
    eends = np.zeros((NCN, K_CH, 128, NE // 16), np.int16)
    for c in range(QC):
        w = ends_ext[:, c].reshape(NCN, K_CH, NE // 16, 16)
        eends[:, :, 16 * c:16 * c + 16, :] = np.swapaxes(w, 2, 3)

    deg = np.bincount(dst, minlength=N).astype(np.float32)
    deg3 = np.zeros((NCN, 3, D_STRIP), np.float32)
    for n in range(NCN):
        deg3[n, :, :NSH] = deg[n * NSH:(n + 1) * NSH][None, :]

    sel = np.zeros((128, 48), np.float32)
    for g in range(QC):
        for f in range(3):
            sel[16 * g + 3 + f, 3 * g + f] = 1.0       # A set: C2/a2 rows
            sel[16 * g + f, 24 + 3 * g + f] = 1.0      # B set: C1/a1 rows

    rep72 = np.zeros((3, 72), np.float32)
    for f in range(3):
        rep72[f, 24 * f:24 * (f + 1)] = 1.0

    W1 = np.ascontiguousarray(np.asarray(inputs["W1"], np.float32))
    wmap = {
        "W1": W1,
        "rep72": rep72,
        "b1c": np.ascontiguousarray(np.asarray(inputs["b1"], np.float32).reshape(HID, 1)),
        "Wl2": np.ascontiguousarray(np.asarray(inputs["Wl2"], np.float32)),
        "Wr2": np.ascontiguousarray(np.asarray(inputs["Wr2"], np.float32)),
        "W2": np.ascontiguousarray(np.asarray(inputs["W2"], np.float32)),
        "Wl1T": np.ascontiguousarray(np.asarray(inputs["Wl1"], np.float32).T),
        "Wr1T": np.ascontiguousarray(np.asarray(inputs["Wr1"], np.float32).T),
        "bl1c": np.ascontiguousarray(np.asarray(inputs["bl1"], np.float32).reshape(HID, 1)),
        "bl2c": np.ascontiguousarray(np.asarray(inputs["bl2"], np.float32).reshape(HID, 1)),
        "b2c": np.ascontiguousarray(np.asarray(inputs["b2"], np.float32).reshape(OUT, 1)),
    }

    in_maps = []
    for n in range(NCN):
        xs = np.zeros((XPAD, IN_F), np.float32)
        xs[:NSH] = x[n * NSH:(n + 1) * NSH]
        m = {
            "x_sh": xs,
            "eidx": np.ascontiguousarray(eidx[n]),
            "eends": np.ascontiguousarray(eends[n]),
            "deg3": np.ascontiguousarray(deg3[n]),
            "selAB": sel,
        }
        m.update(wmap)
        in_maps.append(m)
    return in_maps, C_CAP


def kernel(**inputs) -> np.ndarray:
    from concourse.bass_utils import run_bass_kernel_spmd

    in_maps, C_CAP = _preprocess(inputs)
    if C_CAP not in _prog_cache:
        _prog_cache[C_CAP] = _build_program(C_CAP)
    nc = _prog_cache[C_CAP]
    res = run_bass_kernel_spmd(nc, in_maps, core_ids=list(range(NCN)))
    out = np.empty((N, OUT), np.float32)
    for n in range(NCN):
        out[n * NSH:(n + 1) * NSH] = res.results[n]["outT"].T[:NSH]
    return out



# revision 23
# speedup vs baseline: 1.0040x; 1.0018x over previous
"""Trainium2 Bass kernel for 2-layer GraphSAGE (mean aggregation) on 8 NeuronCores.

Math: with M = mean-aggregation operator (D^-1 A), the reference is
    h  = relu(x @ W1 + b1)
    h1 = (M h) Wl1 + bl1 + h Wr1
    h2 = (M h1) Wl2 + bl2 + h1 Wr2
    out = h2 @ W2 + b2
Everything after the relu is linear, so fold:
    out = M(M(h C2)) + M(h C1) + h C0 + r*c_r + c_1
with C2 = Wl1 Wl2 W2, C1 = (Wr1 Wl2 + Wl1 Wr2) W2, C0 = Wr1 Wr2 W2,
c_r = bl1 Wl2 W2, c_1 = (bl2 + bl1 Wr2) W2 + b2, r = (deg > 0).
The two aggregation passes therefore run on 3-feature vectors.

Distribution: NC n owns src-shard n (12500 nodes; computes h locally).
Its incident edges are grouped by dst-range -> Q7 core, chunked and sorted
by dst.  Per chunk: GPSIMD ap_gather (feature-per-partition tables, int16
src-local indices) -> DVE cumulative-sum scan -> GPSIMD gather at segment
ends -> shifted subtract = per-dst sums.  A PE one-hot matmul compacts the
(group, feature) partitions, and partial sums are ReduceScattered across
the 8 NCs (dst-shard n -> NC n), divided by degree on device, and fed to
pass 2.  Host-side work is only integer graph partitioning / index layout.
"""
import numpy as np

# ---- problem constants (hardcoded per contract) ----
N = 100000
E = 6400000
IN_F = 128
HID = 10
OUT = 3

NCN = 8             # NeuronCores
QC = 8              # Q7 cores per NC
NSH = N // NCN      # 12500 nodes per shard
K_CH = 8            # chunks per (NC, q7core)
D_CHUNK = -(-NSH // K_CH)          # 1042 dsts per chunk
D_STRIP = K_CH * D_CHUNK           # 12504
NE = ((D_CHUNK + 1 + 15) // 16) * 16   # 1056 ends entries per chunk
NSH_TAB = ((NSH + 16 + 15) // 16) * 16  # 12528 table width
SENT = NSH + 6                      # sentinel (zero) table column
XPAD = ((NSH + 511) // 512) * 512   # 12800 padded x rows (512-row groups)
F32 = "float32"

_prog_cache = {}


def _build_program(C_CAP):
    from contextlib import ExitStack
    import concourse.bacc as bacc
    import concourse.tile as tile
    import concourse.mybir as mybir
    from concourse.masks import make_identity

    f32 = mybir.dt.float32
    i16 = mybir.dt.int16
    AL = mybir.AluOpType
    AF = mybir.ActivationFunctionType

    nc = bacc.Bacc("TRN2", target_bir_lowering=False, debug=False,
                   num_devices=NCN)

    # ---- I/O ----
    x_in = nc.dram_tensor("x_sh", [XPAD, IN_F], f32, kind="ExternalInput")
    eidx_in = nc.dram_tensor("eidx", [K_CH, 128, C_CAP // 16], i16, kind="ExternalInput")
    eend_in = nc.dram_tensor("eends", [K_CH, 128, NE // 16], i16, kind="ExternalInput")
    deg_in = nc.dram_tensor("deg3", [3, D_STRIP], f32, kind="ExternalInput")
    sel_in = nc.dram_tensor("selAB", [128, 48], f32, kind="ExternalInput")
    rep72_in = nc.dram_tensor("rep72", [3, 72], f32, kind="ExternalInput")
    W1_in = nc.dram_tensor("W1", [IN_F, HID], f32, kind="ExternalInput")
    b1_in = nc.dram_tensor("b1c", [HID, 1], f32, kind="ExternalInput")
    Wl2_in = nc.dram_tensor("Wl2", [HID, HID], f32, kind="ExternalInput")
    Wr2_in = nc.dram_tensor("Wr2", [HID, HID], f32, kind="ExternalInput")
    W2_in = nc.dram_tensor("W2", [HID, OUT], f32, kind="ExternalInput")
    Wl1T_in = nc.dram_tensor("Wl1T", [HID, HID], f32, kind="ExternalInput")
    Wr1T_in = nc.dram_tensor("Wr1T", [HID, HID], f32, kind="ExternalInput")
    bl1_in = nc.dram_tensor("bl1c", [HID, 1], f32, kind="ExternalInput")
    bl2_in = nc.dram_tensor("bl2c", [HID, 1], f32, kind="ExternalInput")
    b2_in = nc.dram_tensor("b2c", [OUT, 1], f32, kind="ExternalInput")
    out_ext = nc.dram_tensor("outT", [3, D_STRIP], f32, kind="ExternalOutput")

    with tile.TileContext(nc) as tc:
        es = ExitStack()
        with es:
            dram = es.enter_context(tc.tile_pool(name="dram", bufs=1, space="DRAM"))
            p_small = es.enter_context(tc.tile_pool(name="small", bufs=1))

            gc0_d = dram.tile([3, D_STRIP], f32)
            WA = D_STRIP // 24          # 521: [3, D_STRIP] viewed as [72, 521]

            def v72(ap):
                return ap.rearrange("f (a b) -> (f a) b", a=24)
            bounceA = dram.tile([NCN, 3, D_STRIP], f32)
            bounceB = dram.tile([NCN, 3, D_STRIP], f32)
            bounceC = dram.tile([NCN, 3, D_STRIP], f32)
            rsA = dram.tile([3, D_STRIP], f32)
            rsB = dram.tile([3, D_STRIP], f32)
            rsC = dram.tile([3, D_STRIP], f32)

            sel = p_small.tile([128, 48], f32)
            nc.sync.dma_start(out=sel[:], in_=sel_in[:])

            # ---- phase 0: folded weight matrices (tiny PE matmuls) ----
            w1 = p_small.tile([IN_F, HID], f32)
            wl2 = p_small.tile([HID, HID], f32)
            wr2 = p_small.tile([HID, HID], f32)
            w2 = p_small.tile([HID, OUT], f32)
            wl1t = p_small.tile([HID, HID], f32)
            wr1t = p_small.tile([HID, HID], f32)
            b1c = p_small.tile([HID, 1], f32)
            bl1c = p_small.tile([HID, 1], f32)
            bl2c = p_small.tile([HID, 1], f32)
            b2c = p_small.tile([OUT, 1], f32)
            for t, src in [(w1, W1_in), (wl2, Wl2_in), (wr2, Wr2_in),
                           (w2, W2_in), (wl1t, Wl1T_in), (wr1t, Wr1T_in),
                           (b1c, b1_in), (bl1c, bl1_in), (bl2c, bl2_in),
                           (b2c, b2_in)]:
                nc.sync.dma_start(out=t[:], in_=src[:])

            rec_d = dram.tile([3, D_STRIP], f32)
            with tc.tile_pool(name="rec0", bufs=1) as p_rec:
                rt0 = p_rec.tile([72, WA], f32)
                nc.scalar.dma_start(out=rt0[:], in_=v72(deg_in[:]))
                nc.vector.tensor_scalar_max(out=rt0[:], in0=rt0[:], scalar1=1.0)
                nc.vector.reciprocal(out=rt0[:], in_=rt0[:])
                nc.scalar.dma_start(out=v72(rec_d[:]), in_=rt0[:])

            p_ps0 = es.enter_context(tc.tile_pool(name="psum0", bufs=1, space="PSUM"))

            def mm(lhsT, rhs, m, n_, accum=None):
                """matmul into fresh psum, copy to fresh small sbuf tile."""
                ps = p_ps0.tile([m, n_], f32, space="PSUM", tag="ps0")
                if accum is None:
                    nc.tensor.matmul(out=ps[:], lhsT=lhsT, rhs=rhs, start=True, stop=True)
                else:
                    nc.tensor.matmul(out=ps[:], lhsT=lhsT, rhs=rhs, start=True, stop=False)
                    nc.tensor.matmul(out=ps[:], lhsT=accum[0], rhs=accum[1], start=False, stop=True)
                sb = p_small.tile([m, n_], f32, tag=f"mm_{m}x{n_}_{nc.next_id()}")
                nc.vector.tensor_copy(out=sb[:], in_=ps[:])
                return sb

            s2 = mm(wl2[:], wl1t[:], HID, HID)                     # (Wl1 Wl2)^T
            rt = mm(wl2[:], wr1t[:], HID, HID, accum=(wr2[:], wl1t[:]))  # R^T
            s0 = mm(wr2[:], wr1t[:], HID, HID)                     # (Wr1 Wr2)^T
            ccc = p_small.tile([HID, 9], f32)
            for j, lh in [(0, rt), (3, s2), (6, s0)]:
                ps = p_ps0.tile([HID, OUT], f32, space="PSUM", tag="ps0")
                nc.tensor.matmul(out=ps[:], lhsT=lh[:], rhs=w2[:], start=True, stop=True)
                nc.vector.tensor_copy(out=ccc[:, j:j + 3], in_=ps[:])
            t1 = mm(wl2[:], bl1c[:], HID, 1)
            crs = mm(w2[:], t1[:], OUT, 1)                          # c_r [3,1]
            u = mm(wr2[:], bl1c[:], HID, 1)
            nc.vector.tensor_tensor(out=u[:], in0=u[:], in1=bl2c[:], op=AL.add)
            c1s = mm(w2[:], u[:], OUT, 1)                           # pre b2
            nc.vector.tensor_tensor(out=c1s[:], in0=c1s[:], in1=b2c[:], op=AL.add)

            # ---- phase 1: h = relu(x W1 + b1); gc = [hC1 | hC2 | hC0] ----
            p_tab = es.enter_context(tc.tile_pool(name="tab", bufs=1))
            tab = p_tab.tile([128, NSH_TAB], f32)
            nc.scalar.memzero(tab[:])
            with tc.tile_pool(name="lin1", bufs=2) as p_lin, \
                 tc.tile_pool(name="lin1gc", bufs=1) as p_gc, \
                 tc.tile_pool(name="lin1ps", bufs=2, space="PSUM") as p_lps:
                ident = p_small.tile([128, 128], f32)
                make_identity(nc, ident[:])
                gcf = p_gc.tile([9, XPAD], f32)
                n_grp = XPAD // 512
                for g in range(n_grp):
                    xt4 = p_lin.tile([128, 4, 128], f32, tag="xt4")
                    # rows 512g..512g+512 of x -> [p, t, f]
                    nc.sync.dma_start(
                        out=xt4[:],
                        in_=x_in[:].rearrange("(a t p) f -> a p t f", a=XPAD // 512, t=4, p=128)[g])
                    tps = p_lps.tile([128, 512], f32, space="PSUM", tag="tps")
                    for t in range(4):
                        nc.tensor.transpose(out=tps[:, t * 128:(t + 1) * 128],
                                            in_=xt4[:, t, :], identity=ident[:])
                    xtb = p_lin.tile([128, 512], f32, tag="xtb")
                    nc.vector.tensor_copy(out=xtb[:], in_=tps[:])
                    hps = p_lps.tile([HID, 512], f32, space="PSUM", tag="hps")
                    nc.tensor.matmul(out=hps[:], lhsT=w1[:], rhs=xtb[:], start=True, stop=True)
                    hb = p_lin.tile([HID, 512], f32, tag="hb")
                    nc.scalar.activation(out=hb[:], in_=hps[:], func=AF.Relu,
                                         bias=b1c[:], scale=1.0)
                    gps = p_lps.tile([9, 512], f32, space="PSUM", tag="gps")
                    nc.tensor.matmul(out=gps[:], lhsT=ccc[:], rhs=hb[:], start=True, stop=True)
                    nc.vector.tensor_copy(out=gcf[:, g * 512:(g + 1) * 512], in_=gps[:])
                # distribute into gather table (per 16-partition group) + gc0 out
                for g in range(QC):
                    eng = nc.sync if g % 2 == 0 else nc.scalar
                    eng.dma_start(out=tab[16 * g:16 * g + 6, 0:NSH],
                                  in_=gcf[0:6, 0:NSH])
                nc.sync.dma_start(out=gc0_d[:], in_=gcf[6:9, 0:D_STRIP])

            # ---- aggregation passes (software-pipelined on the GPSIMD queue:
            # chunk k+1's main gather is issued before chunk k's ends gather
            # so the Q7 never idles waiting on the DVE scan) ----
            def agg_pass(bounces):
                """bounces: list of (comp_row_offset, dram_view24) to store."""
                with tc.tile_pool(name="agg_msg", bufs=2) as p_msg, \
                     tc.tile_pool(name="agg_sm", bufs=2) as p_asm, \
                     tc.tile_pool(name="agg_ps", bufs=2, space="PSUM") as p_aps:
                    live = {}

                    def front(k):
                        idx_t = p_asm.tile([128, C_CAP // 16], i16, tag="idx")
                        nc.sync.dma_start(out=idx_t[:], in_=eidx_in[k])
                        end_t = p_asm.tile([128, NE // 16], i16, tag="end")
                        nc.sync.dma_start(out=end_t[:], in_=eend_in[k])
                        msg = p_msg.tile([128, C_CAP], f32, tag="msg")
                        nc.gpsimd.ap_gather(
                            out_ap=msg[:], in_ap=tab[:], idxs_ap=idx_t[:],
                            channels=128, num_elems=NSH_TAB, d=1, num_idxs=C_CAP)
                        nc.vector.tensor_tensor_scan(
                            out=msg[:], data0=msg[:], data1=msg[:], initial=0.0,
                            op0=AL.add, op1=AL.bypass)
                        live[k] = (msg, end_t)

                    def back(k):
                        msg, end_t = live.pop(k)
                        gat = p_asm.tile([128, NE], f32, tag="gat")
                        nc.gpsimd.ap_gather(
                            out_ap=gat[:], in_ap=msg[:], idxs_ap=end_t[:],
                            channels=128, num_elems=C_CAP, d=1, num_idxs=NE)
                        strip = p_asm.tile([128, D_CHUNK], f32, tag="strip")
                        nc.vector.tensor_tensor(
                            out=strip[:], in0=gat[:, 1:1 + D_CHUNK],
                            in1=gat[:, 0:D_CHUNK], op=AL.subtract)
                        comp = p_asm.tile([48, D_CHUNK], f32, tag="comp")
                        for j in range(0, D_CHUNK, 512):
                            w = min(512, D_CHUNK - j)
                            cps = p_aps.tile([48, w], f32, space="PSUM", tag="cps")
                            nc.tensor.matmul(out=cps[:], lhsT=sel[:],
                                             rhs=strip[:, j:j + w], start=True, stop=True)
                            nc.vector.tensor_copy(out=comp[:, j:j + w], in_=cps[:])
                        for off, view24 in bounces:
                            nc.sync.dma_start(
                                out=view24[:, k * D_CHUNK:(k + 1) * D_CHUNK],
                                in_=comp[off:off + 24, :])

                    for k in range(K_CH):
                        front(k)
                        if k >= 1:
                            back(k - 1)
                    back(K_CH - 1)

            vA = bounceA[:].rearrange("g f d -> (g f) d")
            vB = bounceB[:].rearrange("g f d -> (g f) d")
            vC = bounceC[:].rearrange("g f d -> (g f) d")

            agg_pass([(0, vA), (24, vB)])

            rg = [list(range(NCN))]
            nc.gpsimd.collective_compute("ReduceScatter", AL.add, replica_groups=rg,
                                         ins=[bounceA.opt()], outs=[rsA.opt()])

            # build pass-2 table: a2' = rsA * recip, replicated per group
            nc.scalar.memzero(tab[:])
            with tc.tile_pool(name="mid", bufs=2) as p_mid:
                ta = p_mid.tile([3, D_STRIP], f32, tag="wide")
                nc.sync.dma_start(out=ta[:], in_=rsA[:])
                td = p_mid.tile([3, D_STRIP], f32, tag="wide")
                nc.scalar.dma_start(out=td[:], in_=rec_d[:])
                nc.vector.tensor_tensor(out=ta[:], in0=ta[:], in1=td[:], op=AL.mult)
                for g in range(QC):
                    eng = nc.sync if g % 2 == 0 else nc.scalar
                    eng.dma_start(out=tab[16 * g:16 * g + 3, 0:D_STRIP], in_=ta[:])

            # RS of the B partials overlaps pass 2 (collectives are async
            # w.r.t. the issuing queue; rsB is only read by final assembly)
            nc.gpsimd.collective_compute("ReduceScatter", AL.add, replica_groups=rg,
                                         ins=[bounceB.opt()], outs=[rsB.opt()])

            agg_pass([(24, vC)])

            nc.gpsimd.collective_compute("ReduceScatter", AL.add, replica_groups=rg,
                                         ins=[bounceC.opt()], outs=[rsC.opt()])

            # replicate [crs | c1s] to 72 partitions for the [72, WA] final math
            rep72 = p_small.tile([3, 72], f32)
            nc.sync.dma_start(out=rep72[:], in_=rep72_in[:])
            cc2 = p_small.tile([3, 2], f32)
            nc.vector.tensor_copy(out=cc2[:, 0:1], in_=crs[:])
            nc.vector.tensor_copy(out=cc2[:, 1:2], in_=c1s[:])
            ps72 = p_ps0.tile([72, 2], f32, space="PSUM", tag="ps0")
            nc.tensor.matmul(out=ps72[:], lhsT=rep72[:], rhs=cc2[:], start=True, stop=True)
            crsc = p_small.tile([72, 2], f32)
            nc.vector.tensor_copy(out=crsc[:], in_=ps72[:])
            # ---- final assembly: out = (a1+b)*recip + gc0 + r*c_r + c_1 ----
            # all elementwise math on a [72, WA] view (24x the lanes of [3, D_STRIP])
            with tc.tile_pool(name="fin", bufs=3) as p_fin:
                # load rsC first: its wait (collective sem >= 3) transitively
                # orders the rsB load behind it on the same queue, closing the
                # window where rsB could be read before ReduceScatter-B lands
                s1 = p_fin.tile([72, WA], f32, tag="fw")
                nc.sync.dma_start(out=s1[:], in_=v72(rsC[:]))
                s2_ = p_fin.tile([72, WA], f32, tag="fw")
                nc.sync.dma_start(out=s2_[:], in_=v72(rsB[:]))
                nc.vector.tensor_tensor(out=s1[:], in0=s1[:], in1=s2_[:], op=AL.add)
                sd = p_fin.tile([72, WA], f32, tag="fw")
                nc.scalar.dma_start(out=sd[:], in_=v72(deg_in[:]))
                sr = p_fin.tile([72, WA], f32, tag="fw")
                nc.sync.dma_start(out=sr[:], in_=v72(rec_d[:]))
                nc.vector.tensor_tensor(out=s1[:], in0=s1[:], in1=sr[:], op=AL.mult)
                # r = (deg > 0); s1 += r * c_r
                nc.vector.tensor_scalar(out=sd[:], in0=sd[:], scalar1=0.0, scalar2=None,
                                        op0=AL.is_gt)
                nc.vector.scalar_tensor_tensor(out=s1[:], in0=sd[:], scalar=crsc[:, 0:1],
                                               in1=s1[:], op0=AL.mult, op1=AL.add)
                sg = p_fin.tile([72, WA], f32, tag="fw")
                nc.sync.dma_start(out=sg[:], in_=v72(gc0_d[:]))
                nc.vector.tensor_tensor(out=s1[:], in0=s1[:], in1=sg[:], op=AL.add)
                nc.scalar.activation(out=s1[:], in_=s1[:], func=AF.Identity,
                                     bias=crsc[:, 1:2], scale=1.0)
                nc.sync.dma_start(out=v72(out_ext[:]), in_=s1[:])

    nc.compile()
    return nc


def _preprocess(inputs):
    """Pure-integer graph partitioning + layout; returns per-core in_maps."""
    x = np.ascontiguousarray(np.asarray(inputs["x"], dtype=np.float32))
    edge_index = np.asarray(inputs["edge_index"])
    src = edge_index[0].astype(np.int64)
    dst = edge_index[1].astype(np.int64)

    n_of_e = src // NSH
    c_of_e = dst // NSH
    k_of_e = (dst % NSH) // D_CHUNK
    d_of_e = (dst % NSH) % D_CHUNK
    src_loc = (src % NSH).astype(np.int16)

    key = ((n_of_e * QC + c_of_e) * K_CH + k_of_e) * np.int64(N) + dst
    order = np.argsort(key, kind="stable")
    so = src_loc[order]
    no = n_of_e[order]
    co = c_of_e[order]
    ko = k_of_e[order]
    do_ = d_of_e[order]

    cell = (no * QC + co) * K_CH + ko
    n_cells = NCN * QC * K_CH
    cnt = np.bincount(cell, minlength=n_cells)
    C_CAP = ((int(cnt.max()) + 1 + 15) // 16) * 16
    cell_start = np.zeros(n_cells + 1, np.int64)
    np.cumsum(cnt, out=cell_start[1:])
    slot = np.arange(E) - cell_start[cell] + 1        # slot 0 = sentinel pad

    eidx = np.full((NCN, K_CH, 128, C_CAP // 16), SENT, np.int16)
    eidx[no, ko, 16 * co + (slot % 16), slot // 16] = so

    dcell = cell * D_CHUNK + do_
    cnt_d = np.bincount(dcell, minlength=n_cells * D_CHUNK).reshape(
        NCN, QC, K_CH, D_CHUNK)
    ce = np.cumsum(cnt_d, axis=3).astype(np.int16)
    ends_ext = np.zeros((NCN, QC, K_CH, NE), np.int16)
    ends_ext[:, :, :, 1:D_CHUNK + 1] = ce
    ends_ext[:, :, :, D_CHUNK + 1:] = ce[:, :, :, -1:]
    eends = np.zeros((NCN, K_CH, 128, NE // 16), np.int16)
    for c in range(QC):
        w = ends_ext[:, c].reshape(NCN, K_CH, NE // 16, 16)
        eends[:, :, 16 * c:16 * c + 16, :] = np.swapaxes(w, 2, 3)

    deg = np.bincount(dst, minlength=N).astype(np.float32)
    deg3 = np.zeros((NCN, 3, D_STRIP), np.float32)
    for n in range(NCN):
        deg3[n, :, :NSH] = deg[n * NSH:(n + 1) * NSH][None, :]

    sel = np.zeros((128, 48), np.float32)
    for g in range(QC):
        for f in range(3):
            sel[16 * g + 3 + f, 3 * g + f] = 1.0       # A set: C2/a2 rows
            sel[16 * g + f, 24 + 3 * g + f] = 1.0      # B set: C1/a1 rows

    rep72 = np.zeros((3, 72), np.float32)
    for f in range(3):
        rep72[f, 24 * f:24 * (f + 1)] = 1.0

    W1 = np.ascontiguousarray(np.asarray(inputs["W1"], np.float32))
    wmap = {
        "W1": W1,
        "rep72": rep72,
        "b1c": np.ascontiguousarray(np.asarray(inputs["b1"], np.float32).reshape(HID, 1)),
        "Wl2": np.ascontiguousarray(np.asarray(inputs["Wl2"], np.float32)),
        "Wr2": np.ascontiguousarray(np.asarray(inputs["Wr2"], np.float32)),
        "W2": np.ascontiguousarray(np.asarray(inputs["W2"], np.float32)),
        "Wl1T": np.ascontiguousarray(np.asarray(inputs["Wl1"], np.float32).T),
        "Wr1T": np.ascontiguousarray(np.asarray(inputs["Wr1"], np.float32).T),
        "bl1c": np.ascontiguousarray(np.asarray(inputs["bl1"], np.float32).reshape(HID, 1)),
        "bl2c": np.ascontiguousarray(np.asarray(inputs["bl2"], np.float32).reshape(HID, 1)),
        "b2c": np.ascontiguousarray(np.asarray(inputs["b2"], np.float32).reshape(OUT, 1)),
    }

    in_maps = []
    for n in range(NCN):
        xs = np.zeros((XPAD, IN_F), np.float32)
        xs[:NSH] = x[n * NSH:(n + 1) * NSH]
        m = {
            "x_sh": xs,
            "eidx": np.ascontiguousarray(eidx[n]),
            "eends": np.ascontiguousarray(eends[n]),
            "deg3": np.ascontiguousarray(deg3[n]),
            "selAB": sel,
        }
        m.update(wmap)
        in_maps.append(m)
    return in_maps, C_CAP


def kernel(**inputs) -> np.ndarray:
    from concourse.bass_utils import run_bass_kernel_spmd

    in_maps, C_CAP = _preprocess(inputs)
    if C_CAP not in _prog_cache:
        _prog_cache[C_CAP] = _build_program(C_CAP)
    nc = _prog_cache[C_CAP]
    res = run_bass_kernel_spmd(nc, in_maps, core_ids=list(range(NCN)))
    out = np.empty((N, OUT), np.float32)
    for n in range(NCN):
        out[n * NSH:(n + 1) * NSH] = res.results[n]["outT"].T[:NSH]
    return out



# revision 24
# speedup vs baseline: 1.0057x; 1.0016x over previous
"""Trainium2 Bass kernel for 2-layer GraphSAGE (mean aggregation) on 8 NeuronCores.

Math: with M = mean-aggregation operator (D^-1 A), the reference is
    h  = relu(x @ W1 + b1)
    h1 = (M h) Wl1 + bl1 + h Wr1
    h2 = (M h1) Wl2 + bl2 + h1 Wr2
    out = h2 @ W2 + b2
Everything after the relu is linear, so fold:
    out = M(M(h C2)) + M(h C1) + h C0 + r*c_r + c_1
with C2 = Wl1 Wl2 W2, C1 = (Wr1 Wl2 + Wl1 Wr2) W2, C0 = Wr1 Wr2 W2,
c_r = bl1 Wl2 W2, c_1 = (bl2 + bl1 Wr2) W2 + b2, r = (deg > 0).
The two aggregation passes therefore run on 3-feature vectors.

Distribution: NC n owns src-shard n (12500 nodes; computes h locally).
Its incident edges are grouped by dst-range -> Q7 core, chunked and sorted
by dst.  Per chunk: GPSIMD ap_gather (feature-per-partition tables, int16
src-local indices) -> DVE cumulative-sum scan -> GPSIMD gather at segment
ends -> shifted subtract = per-dst sums.  A PE one-hot matmul compacts the
(group, feature) partitions, and partial sums are ReduceScattered across
the 8 NCs (dst-shard n -> NC n), divided by degree on device, and fed to
pass 2.  Host-side work is only integer graph partitioning / index layout.
"""
import numpy as np

# ---- problem constants (hardcoded per contract) ----
N = 100000
E = 6400000
IN_F = 128
HID = 10
OUT = 3

NCN = 8             # NeuronCores
QC = 8              # Q7 cores per NC
NSH = N // NCN      # 12500 nodes per shard
K_CH = 8            # chunks per (NC, q7core)
D_CHUNK = -(-NSH // K_CH)          # 1042 dsts per chunk
D_STRIP = K_CH * D_CHUNK           # 12504
NE = ((D_CHUNK + 1 + 15) // 16) * 16   # 1056 ends entries per chunk
NSH_TAB = ((NSH + 16 + 15) // 16) * 16  # 12528 table width
SENT = NSH + 6                      # sentinel (zero) table column
XPAD = ((NSH + 511) // 512) * 512   # 12800 padded x rows (512-row groups)
F32 = "float32"

_prog_cache = {}


def _build_program(C_CAP):
    from contextlib import ExitStack
    import concourse.bacc as bacc
    import concourse.tile as tile
    import concourse.mybir as mybir
    from concourse.masks import make_identity

    f32 = mybir.dt.float32
    i16 = mybir.dt.int16
    AL = mybir.AluOpType
    AF = mybir.ActivationFunctionType

    nc = bacc.Bacc("TRN2", target_bir_lowering=False, debug=False,
                   num_devices=NCN)

    # ---- I/O ----
    x_in = nc.dram_tensor("x_sh", [XPAD, IN_F], f32, kind="ExternalInput")
    eidx_in = nc.dram_tensor("eidx", [K_CH, 128, C_CAP // 16], i16, kind="ExternalInput")
    eend_in = nc.dram_tensor("eends", [K_CH, 128, NE // 16], i16, kind="ExternalInput")
    deg_in = nc.dram_tensor("deg3", [3, D_STRIP], f32, kind="ExternalInput")
    sel_in = nc.dram_tensor("selAB", [128, 48], f32, kind="ExternalInput")
    rep72_in = nc.dram_tensor("rep72", [3, 72], f32, kind="ExternalInput")
    W1_in = nc.dram_tensor("W1", [IN_F, HID], f32, kind="ExternalInput")
    b1_in = nc.dram_tensor("b1c", [HID, 1], f32, kind="ExternalInput")
    Wl2_in = nc.dram_tensor("Wl2", [HID, HID], f32, kind="ExternalInput")
    Wr2_in = nc.dram_tensor("Wr2", [HID, HID], f32, kind="ExternalInput")
    W2_in = nc.dram_tensor("W2", [HID, OUT], f32, kind="ExternalInput")
    Wl1T_in = nc.dram_tensor("Wl1T", [HID, HID], f32, kind="ExternalInput")
    Wr1T_in = nc.dram_tensor("Wr1T", [HID, HID], f32, kind="ExternalInput")
    bl1_in = nc.dram_tensor("bl1c", [HID, 1], f32, kind="ExternalInput")
    bl2_in = nc.dram_tensor("bl2c", [HID, 1], f32, kind="ExternalInput")
    b2_in = nc.dram_tensor("b2c", [OUT, 1], f32, kind="ExternalInput")
    out_ext = nc.dram_tensor("outT", [3, D_STRIP], f32, kind="ExternalOutput")

    with tile.TileContext(nc) as tc:
        es = ExitStack()
        with es:
            dram = es.enter_context(tc.tile_pool(name="dram", bufs=1, space="DRAM"))
            p_small = es.enter_context(tc.tile_pool(name="small", bufs=1))

            gc0_d = dram.tile([3, D_STRIP], f32)
            WA = D_STRIP // 24          # 521: [3, D_STRIP] viewed as [72, 521]

            def v72(ap):
                return ap.rearrange("f (a b) -> (f a) b", a=24)
            bounceA = dram.tile([NCN, 3, D_STRIP], f32)
            bounceB = dram.tile([NCN, 3, D_STRIP], f32)
            bounceC = dram.tile([NCN, 3, D_STRIP], f32)
            rsA = dram.tile([3, D_STRIP], f32)
            rsB = dram.tile([3, D_STRIP], f32)
            rsC = dram.tile([3, D_STRIP], f32)

            sel = p_small.tile([128, 48], f32)
            _padA = p_small.tile([128, 128], f32)   # absolute-shift probe: +512B/partition
            nc.sync.dma_start(out=sel[:], in_=sel_in[:])

            # ---- phase 0: folded weight matrices (tiny PE matmuls) ----
            w1 = p_small.tile([IN_F, HID], f32)
            wl2 = p_small.tile([HID, HID], f32)
            wr2 = p_small.tile([HID, HID], f32)
            w2 = p_small.tile([HID, OUT], f32)
            wl1t = p_small.tile([HID, HID], f32)
            wr1t = p_small.tile([HID, HID], f32)
            b1c = p_small.tile([HID, 1], f32)
            bl1c = p_small.tile([HID, 1], f32)
            bl2c = p_small.tile([HID, 1], f32)
            b2c = p_small.tile([OUT, 1], f32)
            for t, src in [(w1, W1_in), (wl2, Wl2_in), (wr2, Wr2_in),
                           (w2, W2_in), (wl1t, Wl1T_in), (wr1t, Wr1T_in),
                           (b1c, b1_in), (bl1c, bl1_in), (bl2c, bl2_in),
                           (b2c, b2_in)]:
                nc.sync.dma_start(out=t[:], in_=src[:])

            rec_d = dram.tile([3, D_STRIP], f32)
            with tc.tile_pool(name="rec0", bufs=1) as p_rec:
                rt0 = p_rec.tile([72, WA], f32)
                nc.scalar.dma_start(out=rt0[:], in_=v72(deg_in[:]))
                nc.vector.tensor_scalar_max(out=rt0[:], in0=rt0[:], scalar1=1.0)
                nc.vector.reciprocal(out=rt0[:], in_=rt0[:])
                nc.scalar.dma_start(out=v72(rec_d[:]), in_=rt0[:])

            p_ps0 = es.enter_context(tc.tile_pool(name="psum0", bufs=1, space="PSUM"))

            def mm(lhsT, rhs, m, n_, accum=None):
                """matmul into fresh psum, copy to fresh small sbuf tile."""
                ps = p_ps0.tile([m, n_], f32, space="PSUM", tag="ps0")
                if accum is None:
                    nc.tensor.matmul(out=ps[:], lhsT=lhsT, rhs=rhs, start=True, stop=True)
                else:
                    nc.tensor.matmul(out=ps[:], lhsT=lhsT, rhs=rhs, start=True, stop=False)
                    nc.tensor.matmul(out=ps[:], lhsT=accum[0], rhs=accum[1], start=False, stop=True)
                sb = p_small.tile([m, n_], f32, tag=f"mm_{m}x{n_}_{nc.next_id()}")
                nc.vector.tensor_copy(out=sb[:], in_=ps[:])
                return sb

            s2 = mm(wl2[:], wl1t[:], HID, HID)                     # (Wl1 Wl2)^T
            rt = mm(wl2[:], wr1t[:], HID, HID, accum=(wr2[:], wl1t[:]))  # R^T
            s0 = mm(wr2[:], wr1t[:], HID, HID)                     # (Wr1 Wr2)^T
            ccc = p_small.tile([HID, 9], f32)
            for j, lh in [(0, rt), (3, s2), (6, s0)]:
                ps = p_ps0.tile([HID, OUT], f32, space="PSUM", tag="ps0")
                nc.tensor.matmul(out=ps[:], lhsT=lh[:], rhs=w2[:], start=True, stop=True)
                nc.vector.tensor_copy(out=ccc[:, j:j + 3], in_=ps[:])
            t1 = mm(wl2[:], bl1c[:], HID, 1)
            crs = mm(w2[:], t1[:], OUT, 1)                          # c_r [3,1]
            u = mm(wr2[:], bl1c[:], HID, 1)
            nc.vector.tensor_tensor(out=u[:], in0=u[:], in1=bl2c[:], op=AL.add)
            c1s = mm(w2[:], u[:], OUT, 1)                           # pre b2
            nc.vector.tensor_tensor(out=c1s[:], in0=c1s[:], in1=b2c[:], op=AL.add)

            # ---- phase 1: h = relu(x W1 + b1); gc = [hC1 | hC2 | hC0] ----
            p_tab = es.enter_context(tc.tile_pool(name="tab", bufs=1))
            tab = p_tab.tile([128, NSH_TAB], f32)
            nc.scalar.memzero(tab[:])
            with tc.tile_pool(name="lin1", bufs=2) as p_lin, \
                 tc.tile_pool(name="lin1gc", bufs=1) as p_gc, \
                 tc.tile_pool(name="lin1ps", bufs=2, space="PSUM") as p_lps:
                ident = p_small.tile([128, 128], f32)
                make_identity(nc, ident[:])
                gcf = p_gc.tile([9, XPAD], f32)
                n_grp = XPAD // 512
                for g in range(n_grp):
                    xt4 = p_lin.tile([128, 4, 128], f32, tag="xt4")
                    # rows 512g..512g+512 of x -> [p, t, f]
                    nc.sync.dma_start(
                        out=xt4[:],
                        in_=x_in[:].rearrange("(a t p) f -> a p t f", a=XPAD // 512, t=4, p=128)[g])
                    tps = p_lps.tile([128, 512], f32, space="PSUM", tag="tps")
                    for t in range(4):
                        nc.tensor.transpose(out=tps[:, t * 128:(t + 1) * 128],
                                            in_=xt4[:, t, :], identity=ident[:])
                    xtb = p_lin.tile([128, 512], f32, tag="xtb")
                    nc.vector.tensor_copy(out=xtb[:], in_=tps[:])
                    hps = p_lps.tile([HID, 512], f32, space="PSUM", tag="hps")
                    nc.tensor.matmul(out=hps[:], lhsT=w1[:], rhs=xtb[:], start=True, stop=True)
                    hb = p_lin.tile([HID, 512], f32, tag="hb")
                    nc.scalar.activation(out=hb[:], in_=hps[:], func=AF.Relu,
                                         bias=b1c[:], scale=1.0)
                    gps = p_lps.tile([9, 512], f32, space="PSUM", tag="gps")
                    nc.tensor.matmul(out=gps[:], lhsT=ccc[:], rhs=hb[:], start=True, stop=True)
                    nc.vector.tensor_copy(out=gcf[:, g * 512:(g + 1) * 512], in_=gps[:])
                # distribute into gather table (per 16-partition group) + gc0 out
                for g in range(QC):
                    eng = nc.sync if g % 2 == 0 else nc.scalar
                    eng.dma_start(out=tab[16 * g:16 * g + 6, 0:NSH],
                                  in_=gcf[0:6, 0:NSH])
                nc.sync.dma_start(out=gc0_d[:], in_=gcf[6:9, 0:D_STRIP])

            # ---- aggregation passes (software-pipelined on the GPSIMD queue:
            # chunk k+1's main gather is issued before chunk k's ends gather
            # so the Q7 never idles waiting on the DVE scan) ----
            def agg_pass(bounces):
                """bounces: list of (comp_row_offset, dram_view24) to store."""
                with tc.tile_pool(name="agg_msg", bufs=2) as p_msg, \
                     tc.tile_pool(name="agg_sm", bufs=2) as p_asm, \
                     tc.tile_pool(name="agg_ps", bufs=2, space="PSUM") as p_aps:
                    live = {}

                    def front(k):
                        idx_t = p_asm.tile([128, C_CAP // 16], i16, tag="idx")
                        nc.sync.dma_start(out=idx_t[:], in_=eidx_in[k])
                        end_t = p_asm.tile([128, NE // 16], i16, tag="end")
                        nc.sync.dma_start(out=end_t[:], in_=eend_in[k])
                        msg = p_msg.tile([128, C_CAP], f32, tag="msg")
                        nc.gpsimd.ap_gather(
                            out_ap=msg[:], in_ap=tab[:], idxs_ap=idx_t[:],
                            channels=128, num_elems=NSH_TAB, d=1, num_idxs=C_CAP)
                        nc.vector.tensor_tensor_scan(
                            out=msg[:], data0=msg[:], data1=msg[:], initial=0.0,
                            op0=AL.add, op1=AL.bypass)
                        live[k] = (msg, end_t)

                    def back(k):
                        msg, end_t = live.pop(k)
                        gat = p_asm.tile([128, NE], f32, tag="gat")
                        nc.gpsimd.ap_gather(
                            out_ap=gat[:], in_ap=msg[:], idxs_ap=end_t[:],
                            channels=128, num_elems=C_CAP, d=1, num_idxs=NE)
                        strip = p_asm.tile([128, D_CHUNK], f32, tag="strip")
                        nc.vector.tensor_tensor(
                            out=strip[:], in0=gat[:, 1:1 + D_CHUNK],
                            in1=gat[:, 0:D_CHUNK], op=AL.subtract)
                        comp = p_asm.tile([48, D_CHUNK], f32, tag="comp")
                        for j in range(0, D_CHUNK, 512):
                            w = min(512, D_CHUNK - j)
                            cps = p_aps.tile([48, w], f32, space="PSUM", tag="cps")
                            nc.tensor.matmul(out=cps[:], lhsT=sel[:],
                                             rhs=strip[:, j:j + w], start=True, stop=True)
                            nc.vector.tensor_copy(out=comp[:, j:j + w], in_=cps[:])
                        for off, view24 in bounces:
                            nc.sync.dma_start(
                                out=view24[:, k * D_CHUNK:(k + 1) * D_CHUNK],
                                in_=comp[off:off + 24, :])

                    for k in range(K_CH):
                        front(k)
                        if k >= 1:
                            back(k - 1)
                    back(K_CH - 1)

            vA = bounceA[:].rearrange("g f d -> (g f) d")
            vB = bounceB[:].rearrange("g f d -> (g f) d")
            vC = bounceC[:].rearrange("g f d -> (g f) d")

            agg_pass([(0, vA), (24, vB)])

            rg = [list(range(NCN))]
            nc.gpsimd.collective_compute("ReduceScatter", AL.add, replica_groups=rg,
                                         ins=[bounceA.opt()], outs=[rsA.opt()])

            # build pass-2 table: a2' = rsA * recip, replicated per group
            nc.scalar.memzero(tab[:])
            with tc.tile_pool(name="mid", bufs=2) as p_mid:
                ta = p_mid.tile([3, D_STRIP], f32, tag="wide")
                nc.sync.dma_start(out=ta[:], in_=rsA[:])
                td = p_mid.tile([3, D_STRIP], f32, tag="wide")
                nc.scalar.dma_start(out=td[:], in_=rec_d[:])
                nc.vector.tensor_tensor(out=ta[:], in0=ta[:], in1=td[:], op=AL.mult)
                for g in range(QC):
                    eng = nc.sync if g % 2 == 0 else nc.scalar
                    eng.dma_start(out=tab[16 * g:16 * g + 3, 0:D_STRIP], in_=ta[:])

            # RS of the B partials overlaps pass 2 (collectives are async
            # w.r.t. the issuing queue; rsB is only read by final assembly)
            nc.gpsimd.collective_compute("ReduceScatter", AL.add, replica_groups=rg,
                                         ins=[bounceB.opt()], outs=[rsB.opt()])

            agg_pass([(24, vC)])

            nc.gpsimd.collective_compute("ReduceScatter", AL.add, replica_groups=rg,
                                         ins=[bounceC.opt()], outs=[rsC.opt()])

            # replicate [crs | c1s] to 72 partitions for the [72, WA] final math
            rep72 = p_small.tile([3, 72], f32)
            nc.sync.dma_start(out=rep72[:], in_=rep72_in[:])
            cc2 = p_small.tile([3, 2], f32)
            nc.vector.tensor_copy(out=cc2[:, 0:1], in_=crs[:])
            nc.vector.tensor_copy(out=cc2[:, 1:2], in_=c1s[:])
            ps72 = p_ps0.tile([72, 2], f32, space="PSUM", tag="ps0")
            nc.tensor.matmul(out=ps72[:], lhsT=rep72[:], rhs=cc2[:], start=True, stop=True)
            crsc = p_small.tile([72, 2], f32)
            nc.vector.tensor_copy(out=crsc[:], in_=ps72[:])
            # ---- final assembly: out = (a1+b)*recip + gc0 + r*c_r + c_1 ----
            # all elementwise math on a [72, WA] view (24x the lanes of [3, D_STRIP])
            with tc.tile_pool(name="fin", bufs=3) as p_fin:
                # load rsC first: its wait (collective sem >= 3) transitively
                # orders the rsB load behind it on the same queue, closing the
                # window where rsB could be read before ReduceScatter-B lands
                s1 = p_fin.tile([72, WA], f32, tag="fw")
                nc.sync.dma_start(out=s1[:], in_=v72(rsC[:]))
                s2_ = p_fin.tile([72, WA], f32, tag="fw")
                nc.sync.dma_start(out=s2_[:], in_=v72(rsB[:]))
                nc.vector.tensor_tensor(out=s1[:], in0=s1[:], in1=s2_[:], op=AL.add)
                sd = p_fin.tile([72, WA], f32, tag="fw")
                nc.scalar.dma_start(out=sd[:], in_=v72(deg_in[:]))
                sr = p_fin.tile([72, WA], f32, tag="fw")
                nc.sync.dma_start(out=sr[:], in_=v72(rec_d[:]))
                nc.vector.tensor_tensor(out=s1[:], in0=s1[:], in1=sr[:], op=AL.mult)
                # r = (deg > 0); s1 += r * c_r
                nc.vector.tensor_scalar(out=sd[:], in0=sd[:], scalar1=0.0, scalar2=None,
                                        op0=AL.is_gt)
                nc.vector.scalar_tensor_tensor(out=s1[:], in0=sd[:], scalar=crsc[:, 0:1],
                                               in1=s1[:], op0=AL.mult, op1=AL.add)
                sg = p_fin.tile([72, WA], f32, tag="fw")
                nc.sync.dma_start(out=sg[:], in_=v72(gc0_d[:]))
                nc.vector.tensor_tensor(out=s1[:], in0=s1[:], in1=sg[:], op=AL.add)
                nc.scalar.activation(out=s1[:], in_=s1[:], func=AF.Identity,
                                     bias=crsc[:, 1:2], scale=1.0)
                nc.sync.dma_start(out=v72(out_ext[:]), in_=s1[:])

    nc.compile()
    return nc


def _preprocess(inputs):
    """Pure-integer graph partitioning + layout; returns per-core in_maps."""
    x = np.ascontiguousarray(np.asarray(inputs["x"], dtype=np.float32))
    edge_index = np.asarray(inputs["edge_index"])
    src = edge_index[0].astype(np.int64)
    dst = edge_index[1].astype(np.int64)

    n_of_e = src // NSH
    c_of_e = dst // NSH
    k_of_e = (dst % NSH) // D_CHUNK
    d_of_e = (dst % NSH) % D_CHUNK
    src_loc = (src % NSH).astype(np.int16)

    key = ((n_of_e * QC + c_of_e) * K_CH + k_of_e) * np.int64(N) + dst
    order = np.argsort(key, kind="stable")
    so = src_loc[order]
    no = n_of_e[order]
    co = c_of_e[order]
    ko = k_of_e[order]
    do_ = d_of_e[order]

    cell = (no * QC + co) * K_CH + ko
    n_cells = NCN * QC * K_CH
    cnt = np.bincount(cell, minlength=n_cells)
    C_CAP = ((int(cnt.max()) + 1 + 15) // 16) * 16
    cell_start = np.zeros(n_cells + 1, np.int64)
    np.cumsum(cnt, out=cell_start[1:])
    slot = np.arange(E) - cell_start[cell] + 1        # slot 0 = sentinel pad

    eidx = np.full((NCN, K_CH, 128, C_CAP // 16), SENT, np.int16)
    eidx[no, ko, 16 * co + (slot % 16), slot // 16] = so

    dcell = cell * D_CHUNK + do_
    cnt_d = np.bincount(dcell, minlength=n_cells * D_CHUNK).reshape(
        NCN, QC, K_CH, D_CHUNK)
    ce = np.cumsum(cnt_d, axis=3).astype(np.int16)
    ends_ext = np.zeros((NCN, QC, K_CH, NE), np.int16)
    ends_ext[:, :, :, 1:D_CHUNK + 1] = ce
    ends_ext[:, :, :, D_CHUNK + 1:] = ce[:, :, :, -1:]
    eends = np.zeros((NCN, K_CH, 128, NE // 16), np.int16)
    for c in range(QC):
        w = ends_ext[:, c].reshape(NCN, K_CH, NE // 16, 16)
        eends[:, :, 16 * c:16 * c + 16, :] = np.swapaxes(w, 2, 3)

    deg = np.bincount(dst, minlength=N).astype(np.float32)
    deg3 = np.zeros((NCN, 3, D_STRIP), np.float32)
    for n in range(NCN):
        deg3[n, :, :NSH] = deg[n * NSH:(n + 1) * NSH][None, :]

    sel = np.zeros((128, 48), np.float32)
    for g in range(QC):
        for f in range(3):
            sel[16 * g + 3 + f, 3 * g + f] = 1.0       # A set: C2/a2 rows
            sel[16 * g + f, 24 + 3 * g + f] = 1.0      # B set: C1/a1 rows

    rep72 = np.zeros((3, 72), np.float32)
    for f in range(3):
        rep72[f, 24 * f:24 * (f + 1)] = 1.0

    W1 = np.ascontiguousarray(np.asarray(inputs["W1"], np.float32))
    wmap = {
        "W1": W1,
        "rep72": rep72,
        "b1c": np.ascontiguousarray(np.asarray(inputs["b1"], np.float32).reshape(HID, 1)),
        "Wl2": np.ascontiguousarray(np.asarray(inputs["Wl2"], np.float32)),
        "Wr2": np.ascontiguousarray(np.asarray(inputs["Wr2"], np.float32)),
        "W2": np.ascontiguousarray(np.asarray(inputs["W2"], np.float32)),
        "Wl1T": np.ascontiguousarray(np.asarray(inputs["Wl1"], np.float32).T),
        "Wr1T": np.ascontiguousarray(np.asarray(inputs["Wr1"], np.float32).T),
        "bl1c": np.ascontiguousarray(np.asarray(inputs["bl1"], np.float32).reshape(HID, 1)),
        "bl2c": np.ascontiguousarray(np.asarray(inputs["bl2"], np.float32).reshape(HID, 1)),
        "b2c": np.ascontiguousarray(np.asarray(inputs["b2"], np.float32).reshape(OUT, 1)),
    }

    in_maps = []
    for n in range(NCN):
        xs = np.zeros((XPAD, IN_F), np.float32)
        xs[:NSH] = x[n * NSH:(n + 1) * NSH]
        m = {
            "x_sh": xs,
            "eidx": np.ascontiguousarray(eidx[n]),
            "eends": np.ascontiguousarray(eends[n]),
            "deg3": np.ascontiguousarray(deg3[n]),
            "selAB": sel,
        }
        m.update(wmap)
        in_maps.append(m)
    return in_maps, C_CAP


def kernel(**inputs) -> np.ndarray:
    from concourse.bass_utils import run_bass_kernel_spmd

    in_maps, C_CAP = _preprocess(inputs)
    if C_CAP not in _prog_cache:
        _prog_cache[C_CAP] = _build_program(C_CAP)
    nc = _prog_cache[C_CAP]
    res = run_bass_kernel_spmd(nc, in_maps, core_ids=list(range(NCN)))
    out = np.empty((N, OUT), np.float32)
    for n in range(NCN):
        out[n * NSH:(n + 1) * NSH] = res.results[n]["outT"].T[:NSH]
    return out

